# revision 32
# baseline (speedup 1.0000x reference)
# Multi-head attention (RoPE, causal) Trainium2 Bass kernel.
# B=2, S=2048, D=1024, 16 heads, hd=64, fp32 I/O.
#
# Sharding: 32 (batch, head) units over 8 cores -> each core gets one batch
# and 4 heads. Each core computes its 4 heads' attention output and the
# partial out-projection (sum over its heads); the host sums the 4 partials
# per batch and adds the bias constant.
#
# v2: single fused pipeline. QKV tiles (phase A), attention chunks (B) and
# out-projection blocks (C) are interleaved in one tensor-engine stream with
# software pipelining: transposes lag their tile's rope chain by one tile,
# AV matmuls lag their exp by one m-step, so the PE never waits on the
# vector/scalar engines in steady state. Scores are diagonal-trimmed,
# output partials are bf16, and the tail uses a direct reciprocal.
#
# Self-contained: all shapes/sharding hardcoded; no sibling imports.

import numpy as np

import concourse.bass as bass  # noqa: F401
import concourse.mybir as mybir
import concourse.tile as tile
from concourse import bacc, bass_utils

F32 = mybir.dt.float32
BF16 = mybir.dt.bfloat16
EXP = mybir.ActivationFunctionType.Exp

B = 2
S = 2048
D = 1024
NHEADS = 16
HD = 64
HPC = 4  # heads per core
NCORES = 8
NPAIR = 2  # head pairs per core
P = 128
CH = 512  # q chunk
THETA = 10000.0
QKVW = 3 * HPC * HD  # 768

# module-level knobs for test harness
TRACE = False
LAST_RESULTS = None

_PROGRAM_CACHE = {}


def build_program(s=S, mm_fast=True):
    """Build + compile the single-core SPMD program."""
    nt = s // P      # 16 s-tiles
    nch = s // CH    # 4 q chunks
    kt = D // P      # 8 contraction tiles
    nseg = nt // 4   # 4 A-segments
    PD = BF16 if mm_fast else F32

    nc = bacc.Bacc(
        "TRN2", target_bir_lowering=False, debug=False, enable_asserts=False
    )

    # ---- DRAM I/O ----
    xt_d = nc.dram_tensor("xt", [P, kt * s], PD, kind="ExternalInput").ap()
    wt_d = nc.dram_tensor("wt", [P, kt * QKVW], PD, kind="ExternalInput").ap()
    biasqk_d = nc.dram_tensor("biasqk", [P, 512], PD, kind="ExternalInput").ap()
    ropec_d = nc.dram_tensor("ropec", [P, nt * 512], PD, kind="ExternalInput").ap()
    ropes_d = nc.dram_tensor("ropes", [P, nt * 512], PD, kind="ExternalInput").ap()
    trimask_d = nc.dram_tensor("trimask", [P, P], PD, kind="ExternalInput").ap()
    ident_d = nc.dram_tensor("ident", [P, P], PD, kind="ExternalInput").ap()
    wo_d = nc.dram_tensor("wo", [P, NPAIR * D], PD, kind="ExternalInput").ap()
    out_d = nc.dram_tensor("outp", [s, D], PD, kind="ExternalOutput").ap()

    from contextlib import ExitStack

    with tile.TileContext(nc) as tc, ExitStack() as ctx:
        const = ctx.enter_context(tc.tile_pool(name="const", bufs=1))

        # persistent activations
        # qkt2 blocks: [Qpack0, Qpack1, Kpack0, Kpack1] each [128 (2 heads*hd), s]
        qkt2 = const.tile([P, 4 * s], PD)
        # vone: per s-tile [128, 4*65]; per head 64 V cols + ones col
        vone = const.tile([P, nt * (HPC * 65)], PD)
        vone_v = vone.rearrange("p (t h c) -> p t h c", t=nt, h=HPC)
        # otn2: normalized O^T packs: [128 (2 heads*hd), s] per pair
        otn2 = const.tile([P, NPAIR * s], PD)
        # softmax denominators (Z and 1/Z), per (pair, chunk): 1024 cols
        dall = const.tile([1, NPAIR * nch * 1024], F32)
        dallinv = const.tile([1, NPAIR * nch * 1024], PD)
        # half-selector columns for the rank-1 denominator broadcast:
        # halfsel[0, 0:128] selects partitions 0-63, [128:256] selects 64-127
        halfsel = const.tile([1, 256], PD)

        # memsets first (gpsimd)
        nc.gpsimd.memset(vone_v[:, :, :, 64], 1.0)
        nc.gpsimd.memset(halfsel[:, 0:64], 1.0)
        nc.gpsimd.memset(halfsel[:, 64:192], 0.0)
        nc.gpsimd.memset(halfsel[:, 192:256], 1.0)

        # const loads: early ones ride the gpsimd ring (idle at start; the
        # scalar ring must stay free for exp, sync carries the x tiles),
        # later ones are interleaved with the A/B schedule on the sync ring.
        wt_sb = [const.tile([P, 2 * QKVW], PD, name=f"wt{i}") for i in range(4)]
        biasqk_sb = const.tile([P, 512], PD)
        ropec_sb = [const.tile([P, 4 * 512], PD, name=f"ropec{i}") for i in range(nseg)]
        ropes_sb = [const.tile([P, 4 * 512], PD, name=f"ropes{i}") for i in range(nseg)]
        ident_sb = const.tile([P, P], PD)
        trimask_sb = const.tile([P, P], PD)
        wo_sb = const.tile([P, NPAIR * D], PD)

        nc.gpsimd.dma_start(wt_sb[0][:], wt_d[:, 0 : 2 * QKVW])
        nc.gpsimd.dma_start(wt_sb[1][:], wt_d[:, 2 * QKVW : 4 * QKVW])
        nc.gpsimd.dma_start(ident_sb[:], ident_d[:])
        nc.gpsimd.dma_start(biasqk_sb[:], biasqk_d[:])
        nc.gpsimd.dma_start(wt_sb[2][:], wt_d[:, 4 * QKVW : 6 * QKVW])
        nc.gpsimd.dma_start(wt_sb[3][:], wt_d[:, 6 * QKVW : 8 * QKVW])
        nc.gpsimd.dma_start(ropec_sb[0][:], ropec_d[:, 0:2048])
        nc.gpsimd.dma_start(ropes_sb[0][:], ropes_d[:, 0:2048])

        # ---- pools ----
        # PSUM: ring_sc {sc, psA, tp} 2x2 banks, ring_pr {pr, dvb} 2x1 bank
        # (decoupled so pr eviction never gates the next chunk's scores),
        # ot2 single-buffered 2 banks. Total 8 banks.
        pbig = ctx.enter_context(tc.tile_pool(name="pbig", bufs=2, space="PSUM"))
        ppr = ctx.enter_context(tc.tile_pool(name="ppr", bufs=2, space="PSUM"))
        pacc = ctx.enter_context(tc.tile_pool(name="pacc", bufs=1, space="PSUM"))
        xpool = ctx.enter_context(tc.tile_pool(name="xpool", bufs=8))
        aq = ctx.enter_context(tc.tile_pool(name="aq", bufs=2))
        atpool = ctx.enter_context(tc.tile_pool(name="atpool", bufs=3))
        fxw = ctx.enter_context(tc.tile_pool(name="fxw", bufs=2))
        cpool = ctx.enter_context(tc.tile_pool(name="cpool", bufs=3))

        qkt2_v = qkt2.rearrange("p (b s) -> p b s", b=4)

        # ---------------- Phase A: QKV + RoPE (transposes lag 1 tile) -----
        pending_tp = [None]  # (st, rot_tile)

        def flush_tp():
            if pending_tp[0] is None:
                return
            st, rot = pending_tp[0]
            pending_tp[0] = None
            tp = pbig.tile([P, 512], PD, name="tp", tag="big")
            for b in range(4):
                nc.tensor.transpose(
                    tp[:, b * P : (b + 1) * P],
                    rot[:, b * P : (b + 1) * P],
                    ident_sb[:],
                )
            nc.vector.tensor_copy(
                qkt2_v[:, :, st * P : (st + 1) * P],
                tp.rearrange("p (b j) -> p b j", b=4),
            )

        xts_pre = {}

        def preload_x(st):
            xts = xpool.tile([P, D], PD, name="xts")
            nc.sync.dma_start(xts[:], xt_d[:, st * D : (st + 1) * D])
            xts_pre[st] = xts

        def emit_A(st):
            xts = xts_pre.pop(st)
            psA = pbig.tile([P, 1024], F32, name="psA", tag="big")
            for k in range(kt):
                lhs = xts[:, k * P : (k + 1) * P]
                rhs = wt_sb[k // 2][:, (k % 2) * QKVW : (k % 2 + 1) * QKVW]
                nc.tensor.matmul(
                    psA[:, 0:512],
                    lhsT=lhs,
                    rhs=rhs[:, 0:512],
                    start=(k == 0),
                    stop=(k == kt - 1),
                )
                nc.tensor.matmul(
                    psA[:, 512:768],
                    lhsT=lhs,
                    rhs=rhs[:, 512:768],
                    start=(k == 0),
                    stop=(k == kt - 1),
                )
            flush_tp()  # transposes of st-1 go after psA matmuls of st
            # evict V into vone slots (no V bias: folded into host const);
            # scalar is idle during A segments
            nc.scalar.copy(
                vone_v[:, st, :, 0:64],
                psA[:, 512:768].rearrange("p (h c) -> p h c", h=HPC),
            )
            # rope: rot = (qk+b)*cos + swap(qk+b)*sin
            qk = aq.tile([P, 512], PD, name="qk", tag="qk")
            nc.vector.tensor_add(qk[:], psA[:, 0:512], biasqk_sb[:])
            sw = aq.tile([P, 512], PD, name="sw", tag="sw")
            qk_v = qk.rearrange("p (n two) -> p n two", two=2)
            sw_v = sw.rearrange("p (n two) -> p n two", two=2)
            nc.gpsimd.tensor_copy(sw_v[:, :, 0], qk_v[:, :, 1])
            nc.gpsimd.tensor_copy(sw_v[:, :, 1], qk_v[:, :, 0])
            seg, sub = st // 4, st % 4
            rc = ropec_sb[seg][:, sub * 512 : (sub + 1) * 512]
            rs = ropes_sb[seg][:, sub * 512 : (sub + 1) * 512]
            rot = aq.tile([P, 512], PD, name="rot", tag="rot")
            nc.vector.tensor_mul(rot[:], qk[:], rc)
            nc.vector.tensor_mul(sw[:], sw[:], rs)
            nc.vector.tensor_add(rot[:], rot[:], sw[:])
            pending_tp[0] = (st, rot)

        # ---------------- Phase B: attention (AV lags exp by 1 m) --------
        def emit_B(p, j, mid_emit=None):
            q_pack = qkt2[:, p * s : (p + 1) * s]
            k_pack = qkt2[:, (2 + p) * s : (3 + p) * s]
            ot2 = pacc.tile([P, 1024], F32, name="ot2", tag="acc")
            mlast = 4 * j + 3
            qA = q_pack[0:64, j * CH : (j + 1) * CH]
            qB = q_pack[64:128, j * CH : (j + 1) * CH]
            vA = vone_v[:, :, 2 * p, :]
            vB = vone_v[:, :, 2 * p + 1, :]

            def emit_AV(at2, off, m):
                nc.tensor.matmul(
                    ot2[0:65, off:512],
                    lhsT=vA[:, m, :],
                    rhs=at2[:, off:512],
                    start=(m == 0),
                    stop=(m == mlast),
                )
                nc.tensor.matmul(
                    ot2[0:65, 512 + off : 1024],
                    lhsT=vB[:, m, :],
                    rhs=at2[:, 512 + off : 1024],
                    start=(m == 0),
                    stop=(m == mlast),
                )

            prev = None
            for m in range(mlast + 1):
                off = m * P - j * CH if m >= 4 * j else 0
                kA = k_pack[0:64, m * P : (m + 1) * P]
                kB = k_pack[64:128, m * P : (m + 1) * P]
                sc = pbig.tile([P, 1024], F32, name="sc", tag="big")
                nc.tensor.matmul(sc[:, off:512], lhsT=kA, rhs=qA[:, off:512])
                nc.tensor.matmul(
                    sc[:, 512 + off : 1024], lhsT=kB, rhs=qB[:, off:512]
                )
                if prev is not None:
                    emit_AV(*prev)
                at2 = atpool.tile([P, 1024], PD, name="at2", tag="at2")
                if off > 0:
                    sc_v = sc.rearrange("p (h q) -> p h q", h=2)
                    at_v = at2.rearrange("p (h q) -> p h q", h=2)
                    nc.scalar.activation(
                        at_v[:, :, off:512], sc_v[:, :, off:512], EXP, scale=0.125
                    )
                else:
                    nc.scalar.activation(at2[:], sc[:], EXP, scale=0.125)
                if m >= 4 * j:
                    nc.gpsimd.tensor_mul(
                        at2[:, off : off + P], at2[:, off : off + P], trimask_sb[:]
                    )
                    nc.gpsimd.tensor_mul(
                        at2[:, 512 + off : 512 + off + P],
                        at2[:, 512 + off : 512 + off + P],
                        trimask_sb[:],
                    )
                prev = (at2, off, m)
                if m == 1 and mid_emit is not None:
                    mid_emit()
            emit_AV(*prev)

            # ---- fixup: evict O^T halves + denominators, start 1/Z ----
            cs = slice(p * s + j * CH, p * s + (j + 1) * CH)
            nc.vector.tensor_copy(otn2[0:64, cs], ot2[0:64, 0:512])
            stgB = fxw.tile([64, 512], PD, name="stgB", tag="stgB")
            nc.vector.tensor_copy(stgB[:], ot2[0:64, 512:1024])
            nc.sync.dma_start(otn2[64:128, cs], stgB[:])
            dslot = (p * nch + j) * 1024
            nc.vector.tensor_copy(dall[0:1, dslot : dslot + 512], ot2[64:65, 0:512])
            nc.vector.tensor_copy(
                dall[0:1, dslot + 512 : dslot + 1024], ot2[64:65, 512:1024]
            )
            # partition-parallel reciprocal via DMA scatter roundtrip
            dPj = fxw.tile([P, 8], F32, name="dPj", tag="dPj")
            nc.sync.dma_start(
                dPj[:],
                dall[0:1, dslot : dslot + 1024].rearrange("o (a b) -> o a b", a=P),
            )
            dPq = fxw.tile([P, 8], F32, name="dPq", tag="dPq")
            nc.vector.reciprocal(dPq[:], dPj[:])
            dPc = fxw.tile([P, 8], PD, name="dPc", tag="dPc")
            with nc.allow_low_precision("softmax denominators"):
                nc.gpsimd.tensor_copy(dPc[:], dPq[:])
            nc.sync.dma_start(
                dallinv[0:1, dslot : dslot + 1024].rearrange(
                    "o (a b) -> o a b", a=P
                ),
                dPc[:],
            )

        # ---- final: broadcast 1/Z across partitions, normalize in place ----
        def emit_final(p, j):
            dslot = (p * nch + j) * 1024
            dvb = ppr.tile([P, 512], F32, name="dvb", tag="pr")
            nc.tensor.matmul(
                dvb[:],
                lhsT=halfsel[0:1, 0:128],
                rhs=dallinv[0:1, dslot : dslot + 512],
                start=True,
                stop=False,
            )
            nc.tensor.matmul(
                dvb[:],
                lhsT=halfsel[0:1, 128:256],
                rhs=dallinv[0:1, dslot + 512 : dslot + 1024],
                start=False,
                stop=True,
            )
            cs = slice(p * s + j * CH, p * s + (j + 1) * CH)
            nc.vector.tensor_mul(otn2[:, cs], otn2[:, cs], dvb[:])

        # ---------------- Phase C: out projection ----------------
        def emit_C(g):
            for qt in range(4 * g, 4 * g + 4):
                outsb = cpool.tile([P, 1024], PD, name="outsb", tag="outsb")
                for dc in range(2):
                    pr = ppr.tile([P, 512], F32, name="pr", tag="pr")
                    for p in range(NPAIR):
                        nc.tensor.matmul(
                            pr[:],
                            lhsT=otn2[:, p * s + qt * P : p * s + (qt + 1) * P],
                            rhs=wo_sb[:, p * D + dc * 512 : p * D + (dc + 1) * 512],
                            start=(p == 0),
                            stop=(p == NPAIR - 1),
                        )
                    with nc.allow_low_precision("bf16 output partials"):
                        nc.vector.tensor_copy(
                            outsb[:, dc * 512 : (dc + 1) * 512], pr[:]
                        )
                nc.gpsimd.dma_start(out_d[qt * P : (qt + 1) * P, :], outsb[:])

        # ---------------- schedule ----------------
        # x tiles lead the sync ring so the first matmul starts early; const
        # tables slot in behind them. A-tiles run one ahead of the B-chunks
        # that need their transposes so tp flushes never gate.
        for st in range(5):
            preload_x(st)
        emit_A(0)
        nc.sync.dma_start(trimask_sb[:], trimask_d[:])
        emit_A(1)
        emit_A(2)
        nc.sync.dma_start(ropec_sb[1][:], ropec_d[:, 2048:4096])
        nc.sync.dma_start(ropes_sb[1][:], ropes_d[:, 2048:4096])
        emit_A(3)
        emit_A(4)
        for st in range(5, 9):
            preload_x(st)
        nc.sync.dma_start(ropec_sb[2][:], ropec_d[:, 4096:6144])
        nc.sync.dma_start(ropes_sb[2][:], ropes_d[:, 4096:6144])
        BSEQ = [(p, j) for j in range(nch) for p in range(NPAIR)]
        ASEG = {0: [5, 6, 7, 8], 1: [9, 10, 11, 12], 2: [13, 14, 15]}
        LATE = {
            0: ([9, 10, 11, 12], [
                (ropec_sb[3], ropec_d[:, 6144:8192]),
                (ropes_sb[3], ropes_d[:, 6144:8192]),
            ]),
            1: ([13, 14, 15], [(wo_sb, wo_d[:])]),
        }
        pending_final = None
        for idx, (p, j) in enumerate(BSEQ):
            if pending_tp[0] is not None and pending_tp[0][0] <= 4 * j + 3:
                flush_tp()
            mid = None
            if idx == len(BSEQ) - 1 and pending_final is not None:
                pf = pending_final
                pending_final = None

                def mid():
                    emit_final(*pf)

            emit_B(p, j, mid_emit=mid)
            if pending_final is not None:
                emit_final(*pending_final)
                if pending_final[0] == 1:
                    emit_C(pending_final[1])
            pending_final = (p, j)
            if idx in ASEG:
                for st in ASEG[idx]:
                    emit_A(st)
            if idx in LATE:
                pre, lds = LATE[idx]
                for st in pre:
                    preload_x(st)
                for dst, src in lds:
                    nc.sync.dma_start(dst[:], src)
        emit_final(*pending_final)
        emit_C(nch - 1)

    nc.compile()
    return nc


def get_program(s=S, mm_fast=True):
    key = (s, mm_fast)
    if key not in _PROGRAM_CACHE:
        _PROGRAM_CACHE[key] = build_program(s, mm_fast)
    return _PROGRAM_CACHE[key]


def _to_pd(a, mm_fast):
    if mm_fast:
        import ml_dtypes

        return np.ascontiguousarray(a).astype(ml_dtypes.bfloat16)
    return np.ascontiguousarray(a).astype(np.float32)


def prep_core_inputs(x, w_qkv, b_qkv, w_out, core, s=S, mm_fast=True):
    """Build the per-core input map (numpy, host-side sharding/layout)."""
    nt = s // P
    kt = D // P
    b = core // 4
    heads = [(core % 4) * HPC + i for i in range(HPC)]

    xb = np.ascontiguousarray(x[b][:s])  # [s, D]
    # xt[p, st*D + k*128 + j] = x[st*128+j, k*128+p]  (contiguous per s-tile)
    xt = np.ascontiguousarray(
        xb.reshape(nt, P, kt, P).transpose(3, 0, 2, 1).reshape(P, nt * kt * P)
    )

    rows = []
    for part in range(3):
        for h in heads:
            rows.extend(range(part * D + h * HD, part * D + (h + 1) * HD))
    w_sel = w_qkv[rows]  # [768, 1024]
    b_sel = b_qkv[rows]  # [768]
    # wt[p, k*768 + n] = w_sel[n, k*128+p]
    wt = np.ascontiguousarray(
        w_sel.T.reshape(kt, P, QKVW).transpose(1, 0, 2).reshape(P, kt * QKVW)
    )
    biasqk = np.broadcast_to(b_sel[None, 0:512], (P, 512)).copy()

    # rope tables, natural layout per s-tile: [p, st*512 + jj]
    dims = np.arange(0, HD, 2, dtype=np.float64)
    invf = 1.0 / (THETA ** (dims / HD))  # [32]
    pos = np.arange(s, dtype=np.float64)
    ang = pos[:, None] * invf[None, :]  # [s, 32]
    c = np.cos(ang)
    sn = np.sin(ang)
    c2 = np.repeat(c, 2, axis=1)  # [s, 64]
    s2 = np.empty((s, HD))
    s2[:, 0::2] = -sn
    s2[:, 1::2] = sn
    c2h = np.tile(c2, (1, 2 * HPC))  # [s, 512] (Q heads then K heads)
    s2h = np.tile(s2, (1, 2 * HPC))
    ropec = np.ascontiguousarray(
        c2h.reshape(nt, P, 512).transpose(1, 0, 2).reshape(P, nt * 512)
    )
    ropes = np.ascontiguousarray(
        s2h.reshape(nt, P, 512).transpose(1, 0, 2).reshape(P, nt * 512)
    )

    trimask = np.triu(np.ones((P, P), dtype=np.float32))
    ident = np.eye(P, dtype=np.float32)

    # wo[kk, p2*D + n] = w_out[n, gh*64 + kk%64], gh = heads[2*p2 + kk//64]
    wo = np.empty((P, NPAIR * D), dtype=np.float32)
    for p2 in range(NPAIR):
        for half in range(2):
            gh = heads[2 * p2 + half]
            wo[half * 64 : (half + 1) * 64, p2 * D : (p2 + 1) * D] = w_out[
                :, gh * HD : (gh + 1) * HD
            ].T
    return {
        "xt": _to_pd(xt, mm_fast),
        "wt": _to_pd(wt, mm_fast),
        "biasqk": _to_pd(biasqk, mm_fast),
        "ropec": _to_pd(ropec, mm_fast),
        "ropes": _to_pd(ropes, mm_fast),
        "trimask": _to_pd(trimask, mm_fast),
        "ident": _to_pd(ident, mm_fast),
        "wo": _to_pd(wo, mm_fast),
    }


def kernel(x, w_qkv, b_qkv, w_out, b_out, mm_fast=True):
    global LAST_RESULTS
    x = np.asarray(x, dtype=np.float32)
    w_qkv = np.asarray(w_qkv, dtype=np.float32)
    b_qkv = np.asarray(b_qkv, dtype=np.float32)
    w_out = np.asarray(w_out, dtype=np.float32)
    b_out = np.asarray(b_out, dtype=np.float32)

    nc = get_program(mm_fast=mm_fast)
    in_maps = [
        prep_core_inputs(x, w_qkv, b_qkv, w_out, core, mm_fast=mm_fast)
        for core in range(NCORES)
    ]
    res = bass_utils.run_bass_kernel_spmd(
        nc, in_maps, core_ids=list(range(NCORES)), trace=TRACE
    )
    LAST_RESULTS = res
    partials = [r["outp"].astype(np.float32) for r in res.results]
    # v-bias contribution is constant across s (sum_k attn = 1):
    bconst = b_out + b_qkv[2 * D : 3 * D] @ w_out.T
    out = np.stack(
        [
            partials[0] + partials[1] + partials[2] + partials[3],
            partials[4] + partials[5] + partials[6] + partials[7],
        ]
    )
    out = out + bconst[None, None, :]
    return out.astype(np.float32)


# revision 39
# speedup vs baseline: 1.0681x; 1.0681x over previous
# Multi-head attention (RoPE, causal) Trainium2 Bass kernel.
# B=2, S=2048, D=1024, 16 heads, hd=64, fp32 I/O.
#
# Sharding: 32 (batch, head) units over 8 cores -> each core gets one batch
# and 4 heads. Each core computes its 4 heads' attention output and the
# partial out-projection (sum over its heads); the host sums the 4 partials
# per batch and adds the bias constant.
#
# v2: single fused pipeline. QKV tiles (phase A), attention chunks (B) and
# out-projection blocks (C) are interleaved in one tensor-engine stream with
# software pipelining: transposes lag their tile's rope chain by one tile,
# AV matmuls lag their exp by one m-step, so the PE never waits on the
# vector/scalar engines in steady state. Scores are diagonal-trimmed,
# output partials are bf16, and the tail uses a direct reciprocal.
#
# Self-contained: all shapes/sharding hardcoded; no sibling imports.

import numpy as np

import concourse.bass as bass  # noqa: F401
import concourse.mybir as mybir
import concourse.tile as tile
from concourse import bacc, bass_utils

F32 = mybir.dt.float32
BF16 = mybir.dt.bfloat16
EXP = mybir.ActivationFunctionType.Exp

B = 2
S = 2048
D = 1024
NHEADS = 16
HD = 64
HPC = 4  # heads per core
NCORES = 8
NPAIR = 2  # head pairs per core
P = 128
CH = 512  # q chunk
THETA = 10000.0
QKVW = 3 * HPC * HD  # 768

# module-level knobs for test harness
TRACE = False
LAST_RESULTS = None

_PROGRAM_CACHE = {}


def build_program(s=S, mm_fast=True):
    """Build + compile the single-core SPMD program."""
    nt = s // P      # 16 s-tiles
    nch = s // CH    # 4 q chunks
    kt = D // P      # 8 contraction tiles
    nseg = nt // 4   # 4 A-segments
    PD = BF16 if mm_fast else F32

    nc = bacc.Bacc(
        "TRN2", target_bir_lowering=False, debug=False, enable_asserts=False
    )

    # ---- DRAM I/O ----
    xt_d = nc.dram_tensor("xt", [P, kt * s], PD, kind="ExternalInput").ap()
    wt_d = nc.dram_tensor("wt", [P, kt * QKVW], PD, kind="ExternalInput").ap()
    biasqk_d = nc.dram_tensor("biasqk", [P, 512], PD, kind="ExternalInput").ap()
    ropec_d = nc.dram_tensor("ropec", [P, nt * 512], PD, kind="ExternalInput").ap()
    ropes_d = nc.dram_tensor("ropes", [P, nt * 512], PD, kind="ExternalInput").ap()
    trimask_d = nc.dram_tensor("trimask", [P, P], PD, kind="ExternalInput").ap()
    ident_d = nc.dram_tensor("ident", [P, P], PD, kind="ExternalInput").ap()
    wo_d = nc.dram_tensor("wo", [P, NPAIR * D], PD, kind="ExternalInput").ap()
    out_d = nc.dram_tensor("outp", [s, D], PD, kind="ExternalOutput").ap()

    from contextlib import ExitStack

    with tile.TileContext(nc) as tc, ExitStack() as ctx:
        const = ctx.enter_context(tc.tile_pool(name="const", bufs=1))

        # persistent activations
        # qkt2 blocks: [Qpack0, Qpack1, Kpack0, Kpack1] each [128 (2 heads*hd), s]
        qkt2 = const.tile([P, 4 * s], PD)
        # vone: per s-tile [128, 4*65]; per head 64 V cols + ones col
        vone = const.tile([P, nt * (HPC * 65)], PD)
        vone_v = vone.rearrange("p (t h c) -> p t h c", t=nt, h=HPC)
        # otn2: normalized O^T packs: [128 (2 heads*hd), s] per pair
        otn2 = const.tile([P, NPAIR * s], PD)
        # softmax denominators (Z and 1/Z), per (pair, chunk): 1024 cols
        dall = const.tile([1, NPAIR * nch * 1024], F32)
        dallinv = const.tile([1, NPAIR * nch * 1024], PD)
        # half-selector columns for the rank-1 denominator broadcast:
        # halfsel[0, 0:128] selects partitions 0-63, [128:256] selects 64-127
        halfsel = const.tile([1, 256], PD)

        # memsets first (gpsimd)
        nc.gpsimd.memset(vone_v[:, :, :, 64], 1.0)
        nc.gpsimd.memset(halfsel[:, 0:64], 1.0)
        nc.gpsimd.memset(halfsel[:, 64:192], 0.0)
        nc.gpsimd.memset(halfsel[:, 192:256], 1.0)

        # const loads: early ones ride the gpsimd ring (idle at start; the
        # scalar ring must stay free for exp, sync carries the x tiles),
        # later ones are interleaved with the A/B schedule on the sync ring.
        wt_sb = [const.tile([P, 2 * QKVW], PD, name=f"wt{i}") for i in range(4)]
        biasqk_sb = const.tile([P, 512], PD)
        ropec_sb = [const.tile([P, 4 * 512], PD, name=f"ropec{i}") for i in range(nseg)]
        ropes_sb = [const.tile([P, 4 * 512], PD, name=f"ropes{i}") for i in range(nseg)]
        ident_sb = const.tile([P, P], PD)
        trimask_sb = const.tile([P, P], PD)
        wo_sb = const.tile([P, NPAIR * D], PD)

        # early consts on the gpsimd ring (idle at start), later tables on
        # the scalar ring (idle through all of phase A); x tiles own sync
        nc.gpsimd.dma_start(wt_sb[0][:], wt_d[:, 0 : 2 * QKVW])
        nc.gpsimd.dma_start(wt_sb[1][:], wt_d[:, 2 * QKVW : 4 * QKVW])
        nc.gpsimd.dma_start(ident_sb[:], ident_d[:])
        nc.gpsimd.dma_start(biasqk_sb[:], biasqk_d[:])
        nc.gpsimd.dma_start(ropec_sb[0][:], ropec_d[:, 0:2048])
        nc.gpsimd.dma_start(ropes_sb[0][:], ropes_d[:, 0:2048])
        nc.scalar.dma_start(wt_sb[2][:], wt_d[:, 4 * QKVW : 6 * QKVW])
        nc.scalar.dma_start(wt_sb[3][:], wt_d[:, 6 * QKVW : 8 * QKVW])
        for i in range(1, nseg):
            nc.scalar.dma_start(ropec_sb[i][:], ropec_d[:, i * 2048 : (i + 1) * 2048])
            nc.scalar.dma_start(ropes_sb[i][:], ropes_d[:, i * 2048 : (i + 1) * 2048])
        nc.scalar.dma_start(trimask_sb[:], trimask_d[:])
        nc.scalar.dma_start(wo_sb[:], wo_d[:])

        # ---- pools ----
        # PSUM: ring "big" {psA, tp, sc, pr, dvb} 2x2 banks, ot2 2x2 banks
        pbig = ctx.enter_context(tc.tile_pool(name="pbig", bufs=2, space="PSUM"))
        pacc = ctx.enter_context(tc.tile_pool(name="pacc", bufs=2, space="PSUM"))
        xpool = ctx.enter_context(tc.tile_pool(name="xpool", bufs=8))
        aq = ctx.enter_context(tc.tile_pool(name="aq", bufs=2))
        atpool = ctx.enter_context(tc.tile_pool(name="atpool", bufs=3))
        fxw = ctx.enter_context(tc.tile_pool(name="fxw", bufs=2))
        cpool = ctx.enter_context(tc.tile_pool(name="cpool", bufs=3))

        qkt2_v = qkt2.rearrange("p (b s) -> p b s", b=4)

        # ---------------- Phase A: QKV + RoPE (transposes lag 1 tile) -----
        pending_tp = [None]  # (st, rot_tile)

        def flush_tp():
            if pending_tp[0] is None:
                return
            st, rot = pending_tp[0]
            pending_tp[0] = None
            tp = pbig.tile([P, 512], PD, name="tp", tag="big")
            for b in range(4):
                nc.tensor.transpose(
                    tp[:, b * P : (b + 1) * P],
                    rot[:, b * P : (b + 1) * P],
                    ident_sb[:],
                )
            nc.scalar.copy(
                qkt2_v[:, :, st * P : (st + 1) * P],
                tp.rearrange("p (b j) -> p b j", b=4),
            )

        xts_pre = {}

        def preload_x(st):
            xts = xpool.tile([P, D], PD, name="xts")
            nc.sync.dma_start(xts[:], xt_d[:, st * D : (st + 1) * D])
            xts_pre[st] = xts

        def emit_A(st):
            xts = xts_pre.pop(st)
            psA = pbig.tile([P, 1024], F32, name="psA", tag="big")
            for k in range(kt):
                lhs = xts[:, k * P : (k + 1) * P]
                rhs = wt_sb[k // 2][:, (k % 2) * QKVW : (k % 2 + 1) * QKVW]
                nc.tensor.matmul(
                    psA[:, 0:512],
                    lhsT=lhs,
                    rhs=rhs[:, 0:512],
                    start=(k == 0),
                    stop=(k == kt - 1),
                )
                nc.tensor.matmul(
                    psA[:, 512:768],
                    lhsT=lhs,
                    rhs=rhs[:, 512:768],
                    start=(k == 0),
                    stop=(k == kt - 1),
                )
            flush_tp()  # transposes of st-1 go after psA matmuls of st
            # evict V into vone slots (no V bias: folded into host const);
            # scalar is idle during A segments
            nc.scalar.copy(
                vone_v[:, st, :, 0:64],
                psA[:, 512:768].rearrange("p (h c) -> p h c", h=HPC),
            )
            # rope: rot = (qk+b)*cos + swap(qk+b)*sin
            qk = aq.tile([P, 512], PD, name="qk", tag="qk")
            nc.vector.tensor_add(qk[:], psA[:, 0:512], biasqk_sb[:])
            sw = aq.tile([P, 512], PD, name="sw", tag="sw")
            qk_v = qk.rearrange("p (n two) -> p n two", two=2)
            sw_v = sw.rearrange("p (n two) -> p n two", two=2)
            nc.vector.tensor_copy(sw_v[:, :, 0], qk_v[:, :, 1])
            nc.vector.tensor_copy(sw_v[:, :, 1], qk_v[:, :, 0])
            seg, sub = st // 4, st % 4
            rc = ropec_sb[seg][:, sub * 512 : (sub + 1) * 512]
            rs = ropes_sb[seg][:, sub * 512 : (sub + 1) * 512]
            rot = aq.tile([P, 512], PD, name="rot", tag="rot")
            nc.vector.tensor_mul(rot[:], qk[:], rc)
            nc.vector.tensor_mul(sw[:], sw[:], rs)
            nc.vector.tensor_add(rot[:], rot[:], sw[:])
            pending_tp[0] = (st, rot)

        # ---------------- Phase B: attention (AV lags exp by 1 m) --------
        def emit_B(p, j, mid_emit=None):
            q_pack = qkt2[:, p * s : (p + 1) * s]
            k_pack = qkt2[:, (2 + p) * s : (3 + p) * s]
            ot2 = pacc.tile([P, 1024], F32, name="ot2", tag="acc")
            mlast = 4 * j + 3
            qA = q_pack[0:64, j * CH : (j + 1) * CH]
            qB = q_pack[64:128, j * CH : (j + 1) * CH]
            vA = vone_v[:, :, 2 * p, :]
            vB = vone_v[:, :, 2 * p + 1, :]

            def emit_AV(at2, off, m):
                nc.tensor.matmul(
                    ot2[0:65, off:512],
                    lhsT=vA[:, m, :],
                    rhs=at2[:, off:512],
                    start=(m == 0),
                    stop=(m == mlast),
                )
                nc.tensor.matmul(
                    ot2[0:65, 512 + off : 1024],
                    lhsT=vB[:, m, :],
                    rhs=at2[:, 512 + off : 1024],
                    start=(m == 0),
                    stop=(m == mlast),
                )

            prev = None
            for m in range(mlast + 1):
                off = m * P - j * CH if m >= 4 * j else 0
                kA = k_pack[0:64, m * P : (m + 1) * P]
                kB = k_pack[64:128, m * P : (m + 1) * P]
                sc = pbig.tile([P, 1024], F32, name="sc", tag="big")
                nc.tensor.matmul(sc[:, off:512], lhsT=kA, rhs=qA[:, off:512])
                nc.tensor.matmul(
                    sc[:, 512 + off : 1024], lhsT=kB, rhs=qB[:, off:512]
                )
                if prev is not None:
                    emit_AV(*prev)
                at2 = atpool.tile([P, 1024], PD, name="at2", tag="at2")
                if off > 0:
                    sc_v = sc.rearrange("p (h q) -> p h q", h=2)
                    at_v = at2.rearrange("p (h q) -> p h q", h=2)
                    nc.scalar.activation(
                        at_v[:, :, off:512], sc_v[:, :, off:512], EXP, scale=0.125
                    )
                else:
                    nc.scalar.activation(at2[:], sc[:], EXP, scale=0.125)
                if m >= 4 * j:
                    nc.gpsimd.tensor_mul(
                        at2[:, off : off + P], at2[:, off : off + P], trimask_sb[:]
                    )
                    nc.gpsimd.tensor_mul(
                        at2[:, 512 + off : 512 + off + P],
                        at2[:, 512 + off : 512 + off + P],
                        trimask_sb[:],
                    )
                prev = (at2, off, m)
                if m == 1 and mid_emit is not None:
                    mid_emit()
            emit_AV(*prev)

            # ---- fixup: evict O^T halves + denominators, start 1/Z ----
            cs = slice(p * s + j * CH, p * s + (j + 1) * CH)
            nc.vector.tensor_copy(otn2[0:64, cs], ot2[0:64, 0:512])
            stgB = fxw.tile([64, 512], PD, name="stgB", tag="stgB")
            nc.vector.tensor_copy(stgB[:], ot2[0:64, 512:1024])
            nc.sync.dma_start(otn2[64:128, cs], stgB[:])
            dslot = (p * nch + j) * 1024
            nc.vector.tensor_copy(dall[0:1, dslot : dslot + 512], ot2[64:65, 0:512])
            nc.vector.tensor_copy(
                dall[0:1, dslot + 512 : dslot + 1024], ot2[64:65, 512:1024]
            )
            # partition-parallel reciprocal via DMA scatter roundtrip
            dPj = fxw.tile([P, 8], F32, name="dPj", tag="dPj")
            nc.sync.dma_start(
                dPj[:],
                dall[0:1, dslot : dslot + 1024].rearrange("o (a b) -> o a b", a=P),
            )
            dPq = fxw.tile([P, 8], F32, name="dPq", tag="dPq")
            nc.vector.reciprocal(dPq[:], dPj[:])
            dPc = fxw.tile([P, 8], PD, name="dPc", tag="dPc")
            with nc.allow_low_precision("softmax denominators"):
                nc.gpsimd.tensor_copy(dPc[:], dPq[:])
            nc.sync.dma_start(
                dallinv[0:1, dslot : dslot + 1024].rearrange(
                    "o (a b) -> o a b", a=P
                ),
                dPc[:],
            )

        # ---- final: broadcast 1/Z across partitions, normalize in place ----
        def emit_final(p, j):
            dslot = (p * nch + j) * 1024
            dvb = pbig.tile([P, 512], F32, name="dvb", tag="big")
            nc.tensor.matmul(
                dvb[:],
                lhsT=halfsel[0:1, 0:128],
                rhs=dallinv[0:1, dslot : dslot + 512],
                start=True,
                stop=False,
            )
            nc.tensor.matmul(
                dvb[:],
                lhsT=halfsel[0:1, 128:256],
                rhs=dallinv[0:1, dslot + 512 : dslot + 1024],
                start=False,
                stop=True,
            )
            cs = slice(p * s + j * CH, p * s + (j + 1) * CH)
            nc.vector.tensor_mul(otn2[:, cs], otn2[:, cs], dvb[:])

        # ---------------- Phase C: out projection ----------------
        def emit_C(g):
            for qt in range(4 * g, 4 * g + 4):
                pr = pbig.tile([P, 1024], F32, name="pr", tag="big")
                for dc in range(2):
                    for p in range(NPAIR):
                        nc.tensor.matmul(
                            pr[:, dc * 512 : (dc + 1) * 512],
                            lhsT=otn2[:, p * s + qt * P : p * s + (qt + 1) * P],
                            rhs=wo_sb[:, p * D + dc * 512 : p * D + (dc + 1) * 512],
                            start=(p == 0),
                            stop=(p == NPAIR - 1),
                        )
                # split eviction across vector+scalar so neither queue gates
                # the ring handoff for long
                outsb = cpool.tile([P, 1024], PD, name="outsb", tag="outsb")
                with nc.allow_low_precision("bf16 output partials"):
                    nc.vector.tensor_copy(outsb[:, 0:512], pr[:, 0:512])
                    nc.scalar.copy(outsb[:, 512:1024], pr[:, 512:1024])
                eng = nc.sync if qt % 2 == 0 else nc.gpsimd
                eng.dma_start(out_d[qt * P : (qt + 1) * P, :], outsb[:])

        # ---------------- schedule (sequential A, then B/C) ----------------
        for st in range(8):
            preload_x(st)
        for st in range(nt):
            if st == 4:
                for st2 in range(8, 12):
                    preload_x(st2)
            if st == 8:
                for st2 in range(12, 16):
                    preload_x(st2)
            emit_A(st)
        flush_tp()
        BSEQ = [(p, j) for j in range(nch) for p in range(NPAIR)]
        pending_final = None
        for idx, (p, j) in enumerate(BSEQ):
            mid = None
            if idx == len(BSEQ) - 1 and pending_final is not None:
                pf = pending_final
                pending_final = None

                def mid():
                    emit_final(*pf)

            emit_B(p, j, mid_emit=mid)
            if pending_final is not None:
                emit_final(*pending_final)
                if pending_final[0] == 1:
                    emit_C(pending_final[1])
            pending_final = (p, j)
        emit_final(*pending_final)
        emit_C(nch - 1)

    nc.compile()
    return nc


def get_program(s=S, mm_fast=True):
    key = (s, mm_fast)
    if key not in _PROGRAM_CACHE:
        _PROGRAM_CACHE[key] = build_program(s, mm_fast)
    return _PROGRAM_CACHE[key]


def _to_pd(a, mm_fast):
    if mm_fast:
        import ml_dtypes

        return np.ascontiguousarray(a).astype(ml_dtypes.bfloat16)
    return np.ascontiguousarray(a).astype(np.float32)


def prep_core_inputs(x, w_qkv, b_qkv, w_out, core, s=S, mm_fast=True):
    """Build the per-core input map (numpy, host-side sharding/layout)."""
    nt = s // P
    kt = D // P
    b = core // 4
    heads = [(core % 4) * HPC + i for i in range(HPC)]

    xb = np.ascontiguousarray(x[b][:s])  # [s, D]
    # xt[p, st*D + k*128 + j] = x[st*128+j, k*128+p]  (contiguous per s-tile)
    xt = np.ascontiguousarray(
        xb.reshape(nt, P, kt, P).transpose(3, 0, 2, 1).reshape(P, nt * kt * P)
    )

    rows = []
    for part in range(3):
        for h in heads:
            rows.extend(range(part * D + h * HD, part * D + (h + 1) * HD))
    w_sel = w_qkv[rows]  # [768, 1024]
    b_sel = b_qkv[rows]  # [768]
    # wt[p, k*768 + n] = w_sel[n, k*128+p]
    wt = np.ascontiguousarray(
        w_sel.T.reshape(kt, P, QKVW).transpose(1, 0, 2).reshape(P, kt * QKVW)
    )
    biasqk = np.broadcast_to(b_sel[None, 0:512], (P, 512)).copy()

    # rope tables, natural layout per s-tile: [p, st*512 + jj]
    dims = np.arange(0, HD, 2, dtype=np.float64)
    invf = 1.0 / (THETA ** (dims / HD))  # [32]
    pos = np.arange(s, dtype=np.float64)
    ang = pos[:, None] * invf[None, :]  # [s, 32]
    c = np.cos(ang)
    sn = np.sin(ang)
    c2 = np.repeat(c, 2, axis=1)  # [s, 64]
    s2 = np.empty((s, HD))
    s2[:, 0::2] = -sn
    s2[:, 1::2] = sn
    c2h = np.tile(c2, (1, 2 * HPC))  # [s, 512] (Q heads then K heads)
    s2h = np.tile(s2, (1, 2 * HPC))
    ropec = np.ascontiguousarray(
        c2h.reshape(nt, P, 512).transpose(1, 0, 2).reshape(P, nt * 512)
    )
    ropes = np.ascontiguousarray(
        s2h.reshape(nt, P, 512).transpose(1, 0, 2).reshape(P, nt * 512)
    )

    trimask = np.triu(np.ones((P, P), dtype=np.float32))
    ident = np.eye(P, dtype=np.float32)

    # wo[kk, p2*D + n] = w_out[n, gh*64 + kk%64], gh = heads[2*p2 + kk//64]
    wo = np.empty((P, NPAIR * D), dtype=np.float32)
    for p2 in range(NPAIR):
        for half in range(2):
            gh = heads[2 * p2 + half]
            wo[half * 64 : (half + 1) * 64, p2 * D : (p2 + 1) * D] = w_out[
                :, gh * HD : (gh + 1) * HD
            ].T
    return {
        "xt": _to_pd(xt, mm_fast),
        "wt": _to_pd(wt, mm_fast),
        "biasqk": _to_pd(biasqk, mm_fast),
        "ropec": _to_pd(ropec, mm_fast),
        "ropes": _to_pd(ropes, mm_fast),
        "trimask": _to_pd(trimask, mm_fast),
        "ident": _to_pd(ident, mm_fast),
        "wo": _to_pd(wo, mm_fast),
    }


def kernel(x, w_qkv, b_qkv, w_out, b_out, mm_fast=True):
    global LAST_RESULTS
    x = np.asarray(x, dtype=np.float32)
    w_qkv = np.asarray(w_qkv, dtype=np.float32)
    b_qkv = np.asarray(b_qkv, dtype=np.float32)
    w_out = np.asarray(w_out, dtype=np.float32)
    b_out = np.asarray(b_out, dtype=np.float32)

    nc = get_program(mm_fast=mm_fast)
    in_maps = [
        prep_core_inputs(x, w_qkv, b_qkv, w_out, core, mm_fast=mm_fast)
        for core in range(NCORES)
    ]
    res = bass_utils.run_bass_kernel_spmd(
        nc, in_maps, core_ids=list(range(NCORES)), trace=TRACE
    )
    LAST_RESULTS = res
    partials = [r["outp"].astype(np.float32) for r in res.results]
    # v-bias contribution is constant across s (sum_k attn = 1):
    bconst = b_out + b_qkv[2 * D : 3 * D] @ w_out.T
    out = np.stack(
        [
            partials[0] + partials[1] + partials[2] + partials[3],
            partials[4] + partials[5] + partials[6] + partials[7],
        ]
    )
    out = out + bconst[None, None, :]
    return out.astype(np.float32)


# revision 41
# speedup vs baseline: 1.1843x; 1.1088x over previous
# Multi-head attention (RoPE, causal) Trainium2 Bass kernel.
# B=2, S=2048, D=1024, 16 heads, hd=64, fp32 I/O.
#
# Sharding: 32 (batch, head) units over 8 cores -> each core gets one batch
# and 4 heads. Each core computes its 4 heads' attention output and the
# partial out-projection (sum over its heads); the host sums the 4 partials
# per batch and adds the bias constant.
#
# v2: single fused pipeline. QKV tiles (phase A), attention chunks (B) and
# out-projection blocks (C) are interleaved in one tensor-engine stream with
# software pipelining: transposes lag their tile's rope chain by one tile,
# AV matmuls lag their exp by one m-step, so the PE never waits on the
# vector/scalar engines in steady state. Scores are diagonal-trimmed,
# output partials are bf16, and the tail uses a direct reciprocal.
#
# Self-contained: all shapes/sharding hardcoded; no sibling imports.

import numpy as np

import concourse.bass as bass  # noqa: F401
import concourse.mybir as mybir
import concourse.tile as tile
from concourse import bacc, bass_utils

F32 = mybir.dt.float32
BF16 = mybir.dt.bfloat16
EXP = mybir.ActivationFunctionType.Exp

B = 2
S = 2048
D = 1024
NHEADS = 16
HD = 64
HPC = 4  # heads per core
NCORES = 8
NPAIR = 2  # head pairs per core
P = 128
CH = 512  # q chunk
THETA = 10000.0
QKVW = 3 * HPC * HD  # 768

# module-level knobs for test harness
TRACE = False
LAST_RESULTS = None

_PROGRAM_CACHE = {}


def build_program(s=S, mm_fast=True):
    """Build + compile the single-core SPMD program."""
    nt = s // P      # 16 s-tiles
    nch = s // CH    # 4 q chunks
    kt = D // P      # 8 contraction tiles
    nseg = nt // 4   # 4 A-segments
    PD = BF16 if mm_fast else F32

    nc = bacc.Bacc(
        "TRN2", target_bir_lowering=False, debug=False, enable_asserts=False
    )

    # ---- DRAM I/O ----
    xt_d = nc.dram_tensor("xt", [P, kt * s], PD, kind="ExternalInput").ap()
    wt_d = nc.dram_tensor("wt", [P, kt * QKVW], PD, kind="ExternalInput").ap()
    biasqk_d = nc.dram_tensor("biasqk", [P, 512], PD, kind="ExternalInput").ap()
    ropec_d = nc.dram_tensor("ropec", [P, nt * 512], PD, kind="ExternalInput").ap()
    ropes_d = nc.dram_tensor("ropes", [P, nt * 512], PD, kind="ExternalInput").ap()
    trimask_d = nc.dram_tensor("trimask", [P, P], PD, kind="ExternalInput").ap()
    ident_d = nc.dram_tensor("ident", [P, P], PD, kind="ExternalInput").ap()
    wo_d = nc.dram_tensor("wo", [P, NPAIR * D], PD, kind="ExternalInput").ap()
    out_d = nc.dram_tensor("outp", [s, D], PD, kind="ExternalOutput").ap()

    from contextlib import ExitStack

    with tile.TileContext(nc) as tc, ExitStack() as ctx:
        const = ctx.enter_context(tc.tile_pool(name="const", bufs=1))

        # persistent activations
        # qkt2 blocks: [Qpack0, Qpack1, Kpack0, Kpack1] each [128 (2 heads*hd), s]
        qkt2 = const.tile([P, 4 * s], PD)
        # vone: per s-tile [128, 4*65]; per head 64 V cols + ones col
        vone = const.tile([P, nt * (HPC * 65)], PD)
        vone_v = vone.rearrange("p (t h c) -> p t h c", t=nt, h=HPC)
        # otn2: normalized O^T packs: [128 (2 heads*hd), s] per pair
        otn2 = const.tile([P, NPAIR * s], PD)
        # softmax denominators (Z and 1/Z), per (pair, chunk): 1024 cols
        dall = const.tile([1, NPAIR * nch * 1024], F32)
        dallinv = const.tile([1, NPAIR * nch * 1024], PD)
        # half-selector columns for the rank-1 denominator broadcast:
        # halfsel[0, 0:128] selects partitions 0-63, [128:256] selects 64-127
        halfsel = const.tile([1, 256], PD)

        # memsets first (gpsimd)
        nc.gpsimd.memset(vone_v[:, :, :, 64], 1.0)
        nc.gpsimd.memset(halfsel[:, 0:64], 1.0)
        nc.gpsimd.memset(halfsel[:, 64:192], 0.0)
        nc.gpsimd.memset(halfsel[:, 192:256], 1.0)

        # const loads: early ones ride the gpsimd ring (idle at start; the
        # scalar ring must stay free for exp, sync carries the x tiles),
        # later ones are interleaved with the A/B schedule on the sync ring.
        wt_sb = [const.tile([P, 2 * QKVW], PD, name=f"wt{i}") for i in range(4)]
        biasqk_sb = const.tile([P, 512], PD)
        ropec_sb = [const.tile([P, 4 * 512], PD, name=f"ropec{i}") for i in range(nseg)]
        ropes_sb = [const.tile([P, 4 * 512], PD, name=f"ropes{i}") for i in range(nseg)]
        ident_sb = const.tile([P, P], PD)
        trimask_sb = const.tile([P, P], PD)
        wo_sb = const.tile([P, NPAIR * D], PD)

        # early consts on the gpsimd ring (idle at start), later tables on
        # the scalar ring (idle through all of phase A); x tiles own sync
        nc.gpsimd.dma_start(wt_sb[0][:], wt_d[:, 0 : 2 * QKVW])
        nc.gpsimd.dma_start(wt_sb[1][:], wt_d[:, 2 * QKVW : 4 * QKVW])
        nc.gpsimd.dma_start(ident_sb[:], ident_d[:])
        nc.gpsimd.dma_start(biasqk_sb[:], biasqk_d[:])
        nc.gpsimd.dma_start(ropec_sb[0][:], ropec_d[:, 0:2048])
        nc.gpsimd.dma_start(ropes_sb[0][:], ropes_d[:, 0:2048])
        nc.scalar.dma_start(wt_sb[2][:], wt_d[:, 4 * QKVW : 6 * QKVW])
        nc.scalar.dma_start(wt_sb[3][:], wt_d[:, 6 * QKVW : 8 * QKVW])
        for i in range(1, nseg):
            nc.scalar.dma_start(ropec_sb[i][:], ropec_d[:, i * 2048 : (i + 1) * 2048])
            nc.scalar.dma_start(ropes_sb[i][:], ropes_d[:, i * 2048 : (i + 1) * 2048])
        nc.scalar.dma_start(trimask_sb[:], trimask_d[:])
        nc.scalar.dma_start(wo_sb[:], wo_d[:])

        # ---- pools ----
        # PSUM: ring "big" {psA, tp, sc, pr, dvb} 2x2 banks, ot2 2x2 banks
        pbig = ctx.enter_context(tc.tile_pool(name="pbig", bufs=2, space="PSUM"))
        pacc = ctx.enter_context(tc.tile_pool(name="pacc", bufs=2, space="PSUM"))
        xpool = ctx.enter_context(tc.tile_pool(name="xpool", bufs=8))
        aq = ctx.enter_context(tc.tile_pool(name="aq", bufs=2))
        atpool = ctx.enter_context(tc.tile_pool(name="atpool", bufs=3))
        fxw = ctx.enter_context(tc.tile_pool(name="fxw", bufs=2))
        cpool = ctx.enter_context(tc.tile_pool(name="cpool", bufs=3))

        qkt2_v = qkt2.rearrange("p (b s) -> p b s", b=4)

        # ---------------- Phase A: QKV + RoPE (transposes lag 1 tile) -----
        pending_tp = [None]  # (st, rot_tile)

        def flush_tp():
            if pending_tp[0] is None:
                return
            st, rot = pending_tp[0]
            pending_tp[0] = None
            tp = pacc.tile([P, 512], PD, name="tp", tag="acc")
            for b in range(4):
                nc.tensor.transpose(
                    tp[:, b * P : (b + 1) * P],
                    rot[:, b * P : (b + 1) * P],
                    ident_sb[:],
                )
            nc.scalar.copy(
                qkt2_v[:, :, st * P : (st + 1) * P],
                tp.rearrange("p (b j) -> p b j", b=4),
            )

        xts_pre = {}

        def preload_x(st):
            xts = xpool.tile([P, D], PD, name="xts")
            nc.sync.dma_start(xts[:], xt_d[:, st * D : (st + 1) * D])
            xts_pre[st] = xts

        def emit_A(st):
            xts = xts_pre.pop(st)
            psA = pbig.tile([P, 1024], F32, name="psA", tag="big")
            for k in range(kt):
                lhs = xts[:, k * P : (k + 1) * P]
                rhs = wt_sb[k // 2][:, (k % 2) * QKVW : (k % 2 + 1) * QKVW]
                nc.tensor.matmul(
                    psA[:, 0:512],
                    lhsT=lhs,
                    rhs=rhs[:, 0:512],
                    start=(k == 0),
                    stop=(k == kt - 1),
                )
                nc.tensor.matmul(
                    psA[:, 512:768],
                    lhsT=lhs,
                    rhs=rhs[:, 512:768],
                    start=(k == 0),
                    stop=(k == kt - 1),
                )
            flush_tp()  # transposes of st-1 go after psA matmuls of st
            # evict V into vone slots (no V bias: folded into host const);
            # scalar is idle during A segments
            nc.scalar.copy(
                vone_v[:, st, :, 0:64],
                psA[:, 512:768].rearrange("p (h c) -> p h c", h=HPC),
            )
            # rope: rot = (qk+b)*cos + swap(qk+b)*sin
            qk = aq.tile([P, 512], PD, name="qk", tag="qk")
            nc.vector.tensor_add(qk[:], psA[:, 0:512], biasqk_sb[:])
            sw = aq.tile([P, 512], PD, name="sw", tag="sw")
            qk_v = qk.rearrange("p (n two) -> p n two", two=2)
            sw_v = sw.rearrange("p (n two) -> p n two", two=2)
            nc.vector.tensor_copy(sw_v[:, :, 0], qk_v[:, :, 1])
            nc.vector.tensor_copy(sw_v[:, :, 1], qk_v[:, :, 0])
            seg, sub = st // 4, st % 4
            rc = ropec_sb[seg][:, sub * 512 : (sub + 1) * 512]
            rs = ropes_sb[seg][:, sub * 512 : (sub + 1) * 512]
            rot = aq.tile([P, 512], PD, name="rot", tag="rot")
            nc.vector.tensor_mul(rot[:], qk[:], rc)
            nc.vector.tensor_mul(sw[:], sw[:], rs)
            nc.vector.tensor_add(rot[:], rot[:], sw[:])
            pending_tp[0] = (st, rot)

        # ---------------- Phase B: attention (AV lags exp by 1 m) --------
        def emit_B(p, j, mid_emit=None):
            q_pack = qkt2[:, p * s : (p + 1) * s]
            k_pack = qkt2[:, (2 + p) * s : (3 + p) * s]
            ot2 = pacc.tile([P, 1024], F32, name="ot2", tag="acc")
            mlast = 4 * j + 3
            qA = q_pack[0:64, j * CH : (j + 1) * CH]
            qB = q_pack[64:128, j * CH : (j + 1) * CH]
            vA = vone_v[:, :, 2 * p, :]
            vB = vone_v[:, :, 2 * p + 1, :]

            def emit_AV(at2, off, m):
                nc.tensor.matmul(
                    ot2[0:65, off:512],
                    lhsT=vA[:, m, :],
                    rhs=at2[:, off:512],
                    start=(m == 0),
                    stop=(m == mlast),
                )
                nc.tensor.matmul(
                    ot2[0:65, 512 + off : 1024],
                    lhsT=vB[:, m, :],
                    rhs=at2[:, 512 + off : 1024],
                    start=(m == 0),
                    stop=(m == mlast),
                )

            prev = None
            for m in range(mlast + 1):
                off = m * P - j * CH if m >= 4 * j else 0
                kA = k_pack[0:64, m * P : (m + 1) * P]
                kB = k_pack[64:128, m * P : (m + 1) * P]
                sc = pbig.tile([P, 1024], F32, name="sc", tag="big")
                nc.tensor.matmul(sc[:, off:512], lhsT=kA, rhs=qA[:, off:512])
                nc.tensor.matmul(
                    sc[:, 512 + off : 1024], lhsT=kB, rhs=qB[:, off:512]
                )
                if prev is not None:
                    emit_AV(*prev)
                at2 = atpool.tile([P, 1024], PD, name="at2", tag="at2")
                if off > 0:
                    sc_v = sc.rearrange("p (h q) -> p h q", h=2)
                    at_v = at2.rearrange("p (h q) -> p h q", h=2)
                    nc.scalar.activation(
                        at_v[:, :, off:512], sc_v[:, :, off:512], EXP, scale=0.125
                    )
                else:
                    nc.scalar.activation(at2[:], sc[:], EXP, scale=0.125)
                if m >= 4 * j:
                    nc.gpsimd.tensor_mul(
                        at2[:, off : off + P], at2[:, off : off + P], trimask_sb[:]
                    )
                    nc.gpsimd.tensor_mul(
                        at2[:, 512 + off : 512 + off + P],
                        at2[:, 512 + off : 512 + off + P],
                        trimask_sb[:],
                    )
                prev = (at2, off, m)
                if m == 1 and mid_emit is not None:
                    mid_emit()
            emit_AV(*prev)

            # ---- fixup: evict O^T halves + denominators, start 1/Z ----
            cs = slice(p * s + j * CH, p * s + (j + 1) * CH)
            nc.vector.tensor_copy(otn2[0:64, cs], ot2[0:64, 0:512])
            stgB = fxw.tile([64, 512], PD, name="stgB", tag="stgB")
            nc.vector.tensor_copy(stgB[:], ot2[0:64, 512:1024])
            nc.sync.dma_start(otn2[64:128, cs], stgB[:])
            dslot = (p * nch + j) * 1024
            nc.vector.tensor_copy(dall[0:1, dslot : dslot + 512], ot2[64:65, 0:512])
            nc.vector.tensor_copy(
                dall[0:1, dslot + 512 : dslot + 1024], ot2[64:65, 512:1024]
            )
            # partition-parallel reciprocal via DMA scatter roundtrip
            dPj = fxw.tile([P, 8], F32, name="dPj", tag="dPj")
            nc.sync.dma_start(
                dPj[:],
                dall[0:1, dslot : dslot + 1024].rearrange("o (a b) -> o a b", a=P),
            )
            dPq = fxw.tile([P, 8], F32, name="dPq", tag="dPq")
            nc.vector.reciprocal(dPq[:], dPj[:])
            dPc = fxw.tile([P, 8], PD, name="dPc", tag="dPc")
            with nc.allow_low_precision("softmax denominators"):
                nc.gpsimd.tensor_copy(dPc[:], dPq[:])
            nc.sync.dma_start(
                dallinv[0:1, dslot : dslot + 1024].rearrange(
                    "o (a b) -> o a b", a=P
                ),
                dPc[:],
            )

        # ---- final: broadcast 1/Z across partitions, normalize in place ----
        def emit_final(p, j):
            dslot = (p * nch + j) * 1024
            dvb = pacc.tile([P, 512], F32, name="dvb", tag="acc")
            nc.tensor.matmul(
                dvb[:],
                lhsT=halfsel[0:1, 0:128],
                rhs=dallinv[0:1, dslot : dslot + 512],
                start=True,
                stop=False,
            )
            nc.tensor.matmul(
                dvb[:],
                lhsT=halfsel[0:1, 128:256],
                rhs=dallinv[0:1, dslot + 512 : dslot + 1024],
                start=False,
                stop=True,
            )
            cs = slice(p * s + j * CH, p * s + (j + 1) * CH)
            nc.vector.tensor_mul(otn2[:, cs], otn2[:, cs], dvb[:])

        # ---------------- Phase C: out projection ----------------
        def emit_C(g):
            for qt in range(4 * g, 4 * g + 4):
                pr = pbig.tile([P, 1024], F32, name="pr", tag="big")
                for dc in range(2):
                    for p in range(NPAIR):
                        nc.tensor.matmul(
                            pr[:, dc * 512 : (dc + 1) * 512],
                            lhsT=otn2[:, p * s + qt * P : p * s + (qt + 1) * P],
                            rhs=wo_sb[:, p * D + dc * 512 : p * D + (dc + 1) * 512],
                            start=(p == 0),
                            stop=(p == NPAIR - 1),
                        )
                # split eviction across vector+scalar so neither queue gates
                # the ring handoff for long
                outsb = cpool.tile([P, 1024], PD, name="outsb", tag="outsb")
                with nc.allow_low_precision("bf16 output partials"):
                    nc.vector.tensor_copy(outsb[:, 0:512], pr[:, 0:512])
                    nc.scalar.copy(outsb[:, 512:1024], pr[:, 512:1024])
                eng = nc.sync if qt % 2 == 0 else nc.gpsimd
                eng.dma_start(out_d[qt * P : (qt + 1) * P, :], outsb[:])

        # ---------------- schedule (sequential A, then B/C) ----------------
        for st in range(8):
            preload_x(st)
        for st in range(nt):
            if st == 4:
                for st2 in range(8, 12):
                    preload_x(st2)
            if st == 8:
                for st2 in range(12, 16):
                    preload_x(st2)
            emit_A(st)
        flush_tp()
        BSEQ = [(p, j) for j in range(nch) for p in range(NPAIR)]
        pending_final = None
        for idx, (p, j) in enumerate(BSEQ):
            mid = None
            if idx == len(BSEQ) - 1 and pending_final is not None:
                pf = pending_final
                pending_final = None

                def mid():
                    emit_final(*pf)

            emit_B(p, j, mid_emit=mid)
            if pending_final is not None:
                emit_final(*pending_final)
                if pending_final[0] == 1:
                    emit_C(pending_final[1])
            pending_final = (p, j)
        emit_final(*pending_final)
        emit_C(nch - 1)

    nc.compile()
    return nc


def get_program(s=S, mm_fast=True):
    key = (s, mm_fast)
    if key not in _PROGRAM_CACHE:
        _PROGRAM_CACHE[key] = build_program(s, mm_fast)
    return _PROGRAM_CACHE[key]


def _to_pd(a, mm_fast):
    if mm_fast:
        import ml_dtypes

        return np.ascontiguousarray(a).astype(ml_dtypes.bfloat16)
    return np.ascontiguousarray(a).astype(np.float32)


def prep_core_inputs(x, w_qkv, b_qkv, w_out, core, s=S, mm_fast=True):
    """Build the per-core input map (numpy, host-side sharding/layout)."""
    nt = s // P
    kt = D // P
    b = core // 4
    heads = [(core % 4) * HPC + i for i in range(HPC)]

    xb = np.ascontiguousarray(x[b][:s])  # [s, D]
    # xt[p, st*D + k*128 + j] = x[st*128+j, k*128+p]  (contiguous per s-tile)
    xt = np.ascontiguousarray(
        xb.reshape(nt, P, kt, P).transpose(3, 0, 2, 1).reshape(P, nt * kt * P)
    )

    rows = []
    for part in range(3):
        for h in heads:
            rows.extend(range(part * D + h * HD, part * D + (h + 1) * HD))
    w_sel = w_qkv[rows]  # [768, 1024]
    b_sel = b_qkv[rows]  # [768]
    # wt[p, k*768 + n] = w_sel[n, k*128+p]
    wt = np.ascontiguousarray(
        w_sel.T.reshape(kt, P, QKVW).transpose(1, 0, 2).reshape(P, kt * QKVW)
    )
    biasqk = np.broadcast_to(b_sel[None, 0:512], (P, 512)).copy()

    # rope tables, natural layout per s-tile: [p, st*512 + jj]
    dims = np.arange(0, HD, 2, dtype=np.float64)
    invf = 1.0 / (THETA ** (dims / HD))  # [32]
    pos = np.arange(s, dtype=np.float64)
    ang = pos[:, None] * invf[None, :]  # [s, 32]
    c = np.cos(ang)
    sn = np.sin(ang)
    c2 = np.repeat(c, 2, axis=1)  # [s, 64]
    s2 = np.empty((s, HD))
    s2[:, 0::2] = -sn
    s2[:, 1::2] = sn
    c2h = np.tile(c2, (1, 2 * HPC))  # [s, 512] (Q heads then K heads)
    s2h = np.tile(s2, (1, 2 * HPC))
    ropec = np.ascontiguousarray(
        c2h.reshape(nt, P, 512).transpose(1, 0, 2).reshape(P, nt * 512)
    )
    ropes = np.ascontiguousarray(
        s2h.reshape(nt, P, 512).transpose(1, 0, 2).reshape(P, nt * 512)
    )

    trimask = np.triu(np.ones((P, P), dtype=np.float32))
    ident = np.eye(P, dtype=np.float32)

    # wo[kk, p2*D + n] = w_out[n, gh*64 + kk%64], gh = heads[2*p2 + kk//64]
    wo = np.empty((P, NPAIR * D), dtype=np.float32)
    for p2 in range(NPAIR):
        for half in range(2):
            gh = heads[2 * p2 + half]
            wo[half * 64 : (half + 1) * 64, p2 * D : (p2 + 1) * D] = w_out[
                :, gh * HD : (gh + 1) * HD
            ].T
    return {
        "xt": _to_pd(xt, mm_fast),
        "wt": _to_pd(wt, mm_fast),
        "biasqk": _to_pd(biasqk, mm_fast),
        "ropec": _to_pd(ropec, mm_fast),
        "ropes": _to_pd(ropes, mm_fast),
        "trimask": _to_pd(trimask, mm_fast),
        "ident": _to_pd(ident, mm_fast),
        "wo": _to_pd(wo, mm_fast),
    }


def kernel(x, w_qkv, b_qkv, w_out, b_out, mm_fast=True):
    global LAST_RESULTS
    x = np.asarray(x, dtype=np.float32)
    w_qkv = np.asarray(w_qkv, dtype=np.float32)
    b_qkv = np.asarray(b_qkv, dtype=np.float32)
    w_out = np.asarray(w_out, dtype=np.float32)
    b_out = np.asarray(b_out, dtype=np.float32)

    nc = get_program(mm_fast=mm_fast)
    in_maps = [
        prep_core_inputs(x, w_qkv, b_qkv, w_out, core, mm_fast=mm_fast)
        for core in range(NCORES)
    ]
    res = bass_utils.run_bass_kernel_spmd(
        nc, in_maps, core_ids=list(range(NCORES)), trace=TRACE
    )
    LAST_RESULTS = res
    partials = [r["outp"].astype(np.float32) for r in res.results]
    # v-bias contribution is constant across s (sum_k attn = 1):
    bconst = b_out + b_qkv[2 * D : 3 * D] @ w_out.T
    out = np.stack(
        [
            partials[0] + partials[1] + partials[2] + partials[3],
            partials[4] + partials[5] + partials[6] + partials[7],
        ]
    )
    out = out + bconst[None, None, :]
    return out.astype(np.float32)


# revision 42
# speedup vs baseline: 1.2521x; 1.0572x over previous
# Multi-head attention (RoPE, causal) Trainium2 Bass kernel.
# B=2, S=2048, D=1024, 16 heads, hd=64, fp32 I/O.
#
# Sharding: 32 (batch, head) units over 8 cores -> each core gets one batch
# and 4 heads. Each core computes its 4 heads' attention output and the
# partial out-projection (sum over its heads); the host sums the 4 partials
# per batch and adds the bias constant.
#
# v2: single fused pipeline. QKV tiles (phase A), attention chunks (B) and
# out-projection blocks (C) are interleaved in one tensor-engine stream with
# software pipelining: transposes lag their tile's rope chain by one tile,
# AV matmuls lag their exp by one m-step, so the PE never waits on the
# vector/scalar engines in steady state. Scores are diagonal-trimmed,
# output partials are bf16, and the tail uses a direct reciprocal.
#
# Self-contained: all shapes/sharding hardcoded; no sibling imports.

import numpy as np

import concourse.bass as bass  # noqa: F401
import concourse.mybir as mybir
import concourse.tile as tile
from concourse import bacc, bass_utils

F32 = mybir.dt.float32
BF16 = mybir.dt.bfloat16
EXP = mybir.ActivationFunctionType.Exp

B = 2
S = 2048
D = 1024
NHEADS = 16
HD = 64
HPC = 4  # heads per core
NCORES = 8
NPAIR = 2  # head pairs per core
P = 128
CH = 512  # q chunk
THETA = 10000.0
QKVW = 3 * HPC * HD  # 768

# module-level knobs for test harness
TRACE = False
LAST_RESULTS = None

_PROGRAM_CACHE = {}


def build_program(s=S, mm_fast=True):
    """Build + compile the single-core SPMD program."""
    nt = s // P      # 16 s-tiles
    nch = s // CH    # 4 q chunks
    kt = D // P      # 8 contraction tiles
    nseg = nt // 4   # 4 A-segments
    PD = BF16 if mm_fast else F32

    nc = bacc.Bacc(
        "TRN2", target_bir_lowering=False, debug=False, enable_asserts=False
    )

    # ---- DRAM I/O ----
    xt_d = nc.dram_tensor("xt", [P, kt * s], PD, kind="ExternalInput").ap()
    wt_d = nc.dram_tensor("wt", [P, kt * QKVW], PD, kind="ExternalInput").ap()
    biasqk_d = nc.dram_tensor("biasqk", [P, 512], PD, kind="ExternalInput").ap()
    ropec_d = nc.dram_tensor("ropec", [P, nt * 512], PD, kind="ExternalInput").ap()
    ropes_d = nc.dram_tensor("ropes", [P, nt * 512], PD, kind="ExternalInput").ap()
    trimask_d = nc.dram_tensor("trimask", [P, P], PD, kind="ExternalInput").ap()
    ident_d = nc.dram_tensor("ident", [P, P], PD, kind="ExternalInput").ap()
    wo_d = nc.dram_tensor("wo", [P, NPAIR * D], PD, kind="ExternalInput").ap()
    out_d = nc.dram_tensor("outp", [s, D], PD, kind="ExternalOutput").ap()

    from contextlib import ExitStack

    with tile.TileContext(nc) as tc, ExitStack() as ctx:
        const = ctx.enter_context(tc.tile_pool(name="const", bufs=1))

        # persistent activations
        # qkt2 blocks: [Qpack0, Qpack1, Kpack0, Kpack1] each [128 (2 heads*hd), s]
        qkt2 = const.tile([P, 4 * s], PD)
        # vone: per s-tile [128, 4*65]; per head 64 V cols + ones col
        vone = const.tile([P, nt * (HPC * 65)], PD)
        vone_v = vone.rearrange("p (t h c) -> p t h c", t=nt, h=HPC)
        # otn2: normalized O^T packs: [128 (2 heads*hd), s] per pair
        otn2 = const.tile([P, NPAIR * s], PD)
        # softmax denominators (Z and 1/Z), per (pair, chunk): 1024 cols
        dall = const.tile([1, NPAIR * nch * 1024], F32)
        dallinv = const.tile([1, NPAIR * nch * 1024], PD)
        # half-selector columns for the rank-1 denominator broadcast:
        # halfsel[0, 0:128] selects partitions 0-63, [128:256] selects 64-127
        halfsel = const.tile([1, 256], PD)

        # memsets first (gpsimd)
        nc.gpsimd.memset(vone_v[:, :, :, 64], 1.0)
        nc.gpsimd.memset(halfsel[:, 0:64], 1.0)
        nc.gpsimd.memset(halfsel[:, 64:192], 0.0)
        nc.gpsimd.memset(halfsel[:, 192:256], 1.0)

        # const loads: early ones ride the gpsimd ring (idle at start; the
        # scalar ring must stay free for exp, sync carries the x tiles),
        # later ones are interleaved with the A/B schedule on the sync ring.
        wt_sb = [const.tile([P, 2 * QKVW], PD, name=f"wt{i}") for i in range(4)]
        biasqk_sb = const.tile([P, 512], PD)
        ropec_sb = [const.tile([P, 4 * 512], PD, name=f"ropec{i}") for i in range(nseg)]
        ropes_sb = [const.tile([P, 4 * 512], PD, name=f"ropes{i}") for i in range(nseg)]
        ident_sb = const.tile([P, P], PD)
        trimask_sb = const.tile([P, P], PD)
        wo_sb = const.tile([P, NPAIR * D], PD)

        # ALL const loads on the gpsimd ring (gpsimd is idle through phase A;
        # scalar/vector/sync queues must stay clear — a DMA trigger occupies
        # its engine queue for the full transfer). Ordered by first use.
        for i in range(4):
            nc.gpsimd.dma_start(
                wt_sb[i][:], wt_d[:, 2 * i * QKVW : 2 * (i + 1) * QKVW]
            )
        nc.gpsimd.dma_start(ident_sb[:], ident_d[:])
        nc.gpsimd.dma_start(biasqk_sb[:], biasqk_d[:])
        for i in range(nseg):
            nc.gpsimd.dma_start(ropec_sb[i][:], ropec_d[:, i * 2048 : (i + 1) * 2048])
            nc.gpsimd.dma_start(ropes_sb[i][:], ropes_d[:, i * 2048 : (i + 1) * 2048])
        nc.gpsimd.dma_start(trimask_sb[:], trimask_d[:])
        nc.gpsimd.dma_start(wo_sb[:], wo_d[:])

        # ---- pools ----
        # PSUM: ring "big" {psA, tp, sc, pr, dvb} 2x2 banks, ot2 2x2 banks
        pbig = ctx.enter_context(tc.tile_pool(name="pbig", bufs=2, space="PSUM"))
        pacc = ctx.enter_context(tc.tile_pool(name="pacc", bufs=2, space="PSUM"))
        xpool = ctx.enter_context(tc.tile_pool(name="xpool", bufs=8))
        aq = ctx.enter_context(tc.tile_pool(name="aq", bufs=2))
        atpool = ctx.enter_context(tc.tile_pool(name="atpool", bufs=3))
        fxw = ctx.enter_context(tc.tile_pool(name="fxw", bufs=2))
        cpool = ctx.enter_context(tc.tile_pool(name="cpool", bufs=3))

        qkt2_v = qkt2.rearrange("p (b s) -> p b s", b=4)

        # ---------------- Phase A: QKV + RoPE (transposes lag 1 tile) -----
        pending_tp = [None]  # (st, rot_tile)

        def flush_tp():
            if pending_tp[0] is None:
                return
            st, rot = pending_tp[0]
            pending_tp[0] = None
            tp = pacc.tile([P, 512], PD, name="tp", tag="acc")
            for b in range(4):
                nc.tensor.transpose(
                    tp[:, b * P : (b + 1) * P],
                    rot[:, b * P : (b + 1) * P],
                    ident_sb[:],
                )
            nc.scalar.copy(
                qkt2_v[:, :, st * P : (st + 1) * P],
                tp.rearrange("p (b j) -> p b j", b=4),
            )

        xts_pre = {}

        def preload_x(st):
            xts = xpool.tile([P, D], PD, name="xts")
            nc.sync.dma_start(xts[:], xt_d[:, st * D : (st + 1) * D])
            xts_pre[st] = xts

        def emit_A(st):
            xts = xts_pre.pop(st)
            psA = pbig.tile([P, 1024], F32, name="psA", tag="big")
            for k in range(kt):
                lhs = xts[:, k * P : (k + 1) * P]
                rhs = wt_sb[k // 2][:, (k % 2) * QKVW : (k % 2 + 1) * QKVW]
                nc.tensor.matmul(
                    psA[:, 0:512],
                    lhsT=lhs,
                    rhs=rhs[:, 0:512],
                    start=(k == 0),
                    stop=(k == kt - 1),
                )
                nc.tensor.matmul(
                    psA[:, 512:768],
                    lhsT=lhs,
                    rhs=rhs[:, 512:768],
                    start=(k == 0),
                    stop=(k == kt - 1),
                )
            flush_tp()  # transposes of st-1 go after psA matmuls of st
            # evict V into vone slots (no V bias: folded into host const);
            # scalar is idle during A segments
            nc.scalar.copy(
                vone_v[:, st, :, 0:64],
                psA[:, 512:768].rearrange("p (h c) -> p h c", h=HPC),
            )
            # rope: rot = (qk+b)*cos + swap(qk+b)*sin
            qk = aq.tile([P, 512], PD, name="qk", tag="qk")
            nc.vector.tensor_add(qk[:], psA[:, 0:512], biasqk_sb[:])
            sw = aq.tile([P, 512], PD, name="sw", tag="sw")
            qk_v = qk.rearrange("p (n two) -> p n two", two=2)
            sw_v = sw.rearrange("p (n two) -> p n two", two=2)
            nc.vector.tensor_copy(sw_v[:, :, 0], qk_v[:, :, 1])
            nc.vector.tensor_copy(sw_v[:, :, 1], qk_v[:, :, 0])
            seg, sub = st // 4, st % 4
            rc = ropec_sb[seg][:, sub * 512 : (sub + 1) * 512]
            rs = ropes_sb[seg][:, sub * 512 : (sub + 1) * 512]
            rot = aq.tile([P, 512], PD, name="rot", tag="rot")
            nc.vector.tensor_mul(rot[:], qk[:], rc)
            nc.vector.tensor_mul(sw[:], sw[:], rs)
            nc.vector.tensor_add(rot[:], rot[:], sw[:])
            pending_tp[0] = (st, rot)

        # ---------------- Phase B: attention (AV lags exp by 1 m) --------
        def emit_B(p, j, mid_emit=None):
            q_pack = qkt2[:, p * s : (p + 1) * s]
            k_pack = qkt2[:, (2 + p) * s : (3 + p) * s]
            ot2 = pacc.tile([P, 1024], F32, name="ot2", tag="acc")
            mlast = 4 * j + 3
            qA = q_pack[0:64, j * CH : (j + 1) * CH]
            qB = q_pack[64:128, j * CH : (j + 1) * CH]
            vA = vone_v[:, :, 2 * p, :]
            vB = vone_v[:, :, 2 * p + 1, :]

            def emit_AV(at2, off, m):
                nc.tensor.matmul(
                    ot2[0:65, off:512],
                    lhsT=vA[:, m, :],
                    rhs=at2[:, off:512],
                    start=(m == 0),
                    stop=(m == mlast),
                )
                nc.tensor.matmul(
                    ot2[0:65, 512 + off : 1024],
                    lhsT=vB[:, m, :],
                    rhs=at2[:, 512 + off : 1024],
                    start=(m == 0),
                    stop=(m == mlast),
                )

            prev = None
            for m in range(mlast + 1):
                off = m * P - j * CH if m >= 4 * j else 0
                kA = k_pack[0:64, m * P : (m + 1) * P]
                kB = k_pack[64:128, m * P : (m + 1) * P]
                sc = pbig.tile([P, 1024], F32, name="sc", tag="big")
                nc.tensor.matmul(sc[:, off:512], lhsT=kA, rhs=qA[:, off:512])
                nc.tensor.matmul(
                    sc[:, 512 + off : 1024], lhsT=kB, rhs=qB[:, off:512]
                )
                if prev is not None:
                    emit_AV(*prev)
                at2 = atpool.tile([P, 1024], PD, name="at2", tag="at2")
                if off > 0:
                    sc_v = sc.rearrange("p (h q) -> p h q", h=2)
                    at_v = at2.rearrange("p (h q) -> p h q", h=2)
                    nc.scalar.activation(
                        at_v[:, :, off:512], sc_v[:, :, off:512], EXP, scale=0.125
                    )
                else:
                    nc.scalar.activation(at2[:], sc[:], EXP, scale=0.125)
                if m >= 4 * j:
                    nc.gpsimd.tensor_mul(
                        at2[:, off : off + P], at2[:, off : off + P], trimask_sb[:]
                    )
                    nc.gpsimd.tensor_mul(
                        at2[:, 512 + off : 512 + off + P],
                        at2[:, 512 + off : 512 + off + P],
                        trimask_sb[:],
                    )
                prev = (at2, off, m)
                if m == 1 and mid_emit is not None:
                    mid_emit()
            emit_AV(*prev)

            # ---- fixup: evict O^T halves + denominators, start 1/Z ----
            cs = slice(p * s + j * CH, p * s + (j + 1) * CH)
            nc.vector.tensor_copy(otn2[0:64, cs], ot2[0:64, 0:512])
            stgB = fxw.tile([64, 512], PD, name="stgB", tag="stgB")
            nc.vector.tensor_copy(stgB[:], ot2[0:64, 512:1024])
            nc.sync.dma_start(otn2[64:128, cs], stgB[:])
            dslot = (p * nch + j) * 1024
            nc.vector.tensor_copy(dall[0:1, dslot : dslot + 512], ot2[64:65, 0:512])
            nc.vector.tensor_copy(
                dall[0:1, dslot + 512 : dslot + 1024], ot2[64:65, 512:1024]
            )
            # partition-parallel reciprocal via DMA scatter roundtrip
            dPj = fxw.tile([P, 8], F32, name="dPj", tag="dPj")
            nc.sync.dma_start(
                dPj[:],
                dall[0:1, dslot : dslot + 1024].rearrange("o (a b) -> o a b", a=P),
            )
            dPq = fxw.tile([P, 8], F32, name="dPq", tag="dPq")
            nc.vector.reciprocal(dPq[:], dPj[:])
            dPc = fxw.tile([P, 8], PD, name="dPc", tag="dPc")
            with nc.allow_low_precision("softmax denominators"):
                nc.gpsimd.tensor_copy(dPc[:], dPq[:])
            nc.sync.dma_start(
                dallinv[0:1, dslot : dslot + 1024].rearrange(
                    "o (a b) -> o a b", a=P
                ),
                dPc[:],
            )

        # ---- final: broadcast 1/Z across partitions, normalize in place ----
        def emit_final(p, j):
            dslot = (p * nch + j) * 1024
            dvb = pacc.tile([P, 512], F32, name="dvb", tag="acc")
            nc.tensor.matmul(
                dvb[:],
                lhsT=halfsel[0:1, 0:128],
                rhs=dallinv[0:1, dslot : dslot + 512],
                start=True,
                stop=False,
            )
            nc.tensor.matmul(
                dvb[:],
                lhsT=halfsel[0:1, 128:256],
                rhs=dallinv[0:1, dslot + 512 : dslot + 1024],
                start=False,
                stop=True,
            )
            cs = slice(p * s + j * CH, p * s + (j + 1) * CH)
            nc.vector.tensor_mul(otn2[:, cs], otn2[:, cs], dvb[:])

        # ---------------- Phase C: out projection ----------------
        def emit_C(g):
            for qt in range(4 * g, 4 * g + 4):
                pr = pbig.tile([P, 1024], F32, name="pr", tag="big")
                for dc in range(2):
                    for p in range(NPAIR):
                        nc.tensor.matmul(
                            pr[:, dc * 512 : (dc + 1) * 512],
                            lhsT=otn2[:, p * s + qt * P : p * s + (qt + 1) * P],
                            rhs=wo_sb[:, p * D + dc * 512 : p * D + (dc + 1) * 512],
                            start=(p == 0),
                            stop=(p == NPAIR - 1),
                        )
                # split eviction across vector+scalar so neither queue gates
                # the ring handoff for long
                outsb = cpool.tile([P, 1024], PD, name="outsb", tag="outsb")
                with nc.allow_low_precision("bf16 output partials"):
                    nc.vector.tensor_copy(outsb[:, 0:512], pr[:, 0:512])
                    nc.scalar.copy(outsb[:, 512:1024], pr[:, 512:1024])
                eng = nc.sync if qt % 2 == 0 else nc.gpsimd
                eng.dma_start(out_d[qt * P : (qt + 1) * P, :], outsb[:])

        # ---------------- schedule (sequential A, then B/C) ----------------
        for st in range(8):
            preload_x(st)
        for st in range(nt):
            if st == 4:
                for st2 in range(8, 12):
                    preload_x(st2)
            if st == 8:
                for st2 in range(12, 16):
                    preload_x(st2)
            emit_A(st)
        flush_tp()
        BSEQ = [(p, j) for j in range(nch) for p in range(NPAIR)]
        pending_final = None
        for idx, (p, j) in enumerate(BSEQ):
            mid = None
            if idx == len(BSEQ) - 1 and pending_final is not None:
                pf = pending_final
                pending_final = None

                def mid():
                    emit_final(*pf)

            emit_B(p, j, mid_emit=mid)
            if pending_final is not None:
                emit_final(*pending_final)
                if pending_final[0] == 1:
                    emit_C(pending_final[1])
            pending_final = (p, j)
        emit_final(*pending_final)
        emit_C(nch - 1)

    nc.compile()
    return nc


def get_program(s=S, mm_fast=True):
    key = (s, mm_fast)
    if key not in _PROGRAM_CACHE:
        _PROGRAM_CACHE[key] = build_program(s, mm_fast)
    return _PROGRAM_CACHE[key]


def _to_pd(a, mm_fast):
    if mm_fast:
        import ml_dtypes

        return np.ascontiguousarray(a).astype(ml_dtypes.bfloat16)
    return np.ascontiguousarray(a).astype(np.float32)


def prep_core_inputs(x, w_qkv, b_qkv, w_out, core, s=S, mm_fast=True):
    """Build the per-core input map (numpy, host-side sharding/layout)."""
    nt = s // P
    kt = D // P
    b = core // 4
    heads = [(core % 4) * HPC + i for i in range(HPC)]

    xb = np.ascontiguousarray(x[b][:s])  # [s, D]
    # xt[p, st*D + k*128 + j] = x[st*128+j, k*128+p]  (contiguous per s-tile)
    xt = np.ascontiguousarray(
        xb.reshape(nt, P, kt, P).transpose(3, 0, 2, 1).reshape(P, nt * kt * P)
    )

    rows = []
    for part in range(3):
        for h in heads:
            rows.extend(range(part * D + h * HD, part * D + (h + 1) * HD))
    w_sel = w_qkv[rows]  # [768, 1024]
    b_sel = b_qkv[rows]  # [768]
    # wt[p, k*768 + n] = w_sel[n, k*128+p]
    wt = np.ascontiguousarray(
        w_sel.T.reshape(kt, P, QKVW).transpose(1, 0, 2).reshape(P, kt * QKVW)
    )
    biasqk = np.broadcast_to(b_sel[None, 0:512], (P, 512)).copy()

    # rope tables, natural layout per s-tile: [p, st*512 + jj]
    dims = np.arange(0, HD, 2, dtype=np.float64)
    invf = 1.0 / (THETA ** (dims / HD))  # [32]
    pos = np.arange(s, dtype=np.float64)
    ang = pos[:, None] * invf[None, :]  # [s, 32]
    c = np.cos(ang)
    sn = np.sin(ang)
    c2 = np.repeat(c, 2, axis=1)  # [s, 64]
    s2 = np.empty((s, HD))
    s2[:, 0::2] = -sn
    s2[:, 1::2] = sn
    c2h = np.tile(c2, (1, 2 * HPC))  # [s, 512] (Q heads then K heads)
    s2h = np.tile(s2, (1, 2 * HPC))
    ropec = np.ascontiguousarray(
        c2h.reshape(nt, P, 512).transpose(1, 0, 2).reshape(P, nt * 512)
    )
    ropes = np.ascontiguousarray(
        s2h.reshape(nt, P, 512).transpose(1, 0, 2).reshape(P, nt * 512)
    )

    trimask = np.triu(np.ones((P, P), dtype=np.float32))
    ident = np.eye(P, dtype=np.float32)

    # wo[kk, p2*D + n] = w_out[n, gh*64 + kk%64], gh = heads[2*p2 + kk//64]
    wo = np.empty((P, NPAIR * D), dtype=np.float32)
    for p2 in range(NPAIR):
        for half in range(2):
            gh = heads[2 * p2 + half]
            wo[half * 64 : (half + 1) * 64, p2 * D : (p2 + 1) * D] = w_out[
                :, gh * HD : (gh + 1) * HD
            ].T
    return {
        "xt": _to_pd(xt, mm_fast),
        "wt": _to_pd(wt, mm_fast),
        "biasqk": _to_pd(biasqk, mm_fast),
        "ropec": _to_pd(ropec, mm_fast),
        "ropes": _to_pd(ropes, mm_fast),
        "trimask": _to_pd(trimask, mm_fast),
        "ident": _to_pd(ident, mm_fast),
        "wo": _to_pd(wo, mm_fast),
    }


def kernel(x, w_qkv, b_qkv, w_out, b_out, mm_fast=True):
    global LAST_RESULTS
    x = np.asarray(x, dtype=np.float32)
    w_qkv = np.asarray(w_qkv, dtype=np.float32)
    b_qkv = np.asarray(b_qkv, dtype=np.float32)
    w_out = np.asarray(w_out, dtype=np.float32)
    b_out = np.asarray(b_out, dtype=np.float32)

    nc = get_program(mm_fast=mm_fast)
    in_maps = [
        prep_core_inputs(x, w_qkv, b_qkv, w_out, core, mm_fast=mm_fast)
        for core in range(NCORES)
    ]
    res = bass_utils.run_bass_kernel_spmd(
        nc, in_maps, core_ids=list(range(NCORES)), trace=TRACE
    )
    LAST_RESULTS = res
    partials = [r["outp"].astype(np.float32) for r in res.results]
    # v-bias contribution is constant across s (sum_k attn = 1):
    bconst = b_out + b_qkv[2 * D : 3 * D] @ w_out.T
    out = np.stack(
        [
            partials[0] + partials[1] + partials[2] + partials[3],
            partials[4] + partials[5] + partials[6] + partials[7],
        ]
    )
    out = out + bconst[None, None, :]
    return out.astype(np.float32)


# revision 45
# speedup vs baseline: 1.3056x; 1.0427x over previous
# Multi-head attention (RoPE, causal) Trainium2 Bass kernel.
# B=2, S=2048, D=1024, 16 heads, hd=64, fp32 I/O.
#
# Sharding: 32 (batch, head) units over 8 cores -> each core gets one batch
# and 4 heads. Each core computes its 4 heads' attention output and the
# partial out-projection (sum over its heads); the host sums the 4 partials
# per batch and adds the bias constant.
#
# v2: single fused pipeline. QKV tiles (phase A), attention chunks (B) and
# out-projection blocks (C) are interleaved in one tensor-engine stream with
# software pipelining: transposes lag their tile's rope chain by one tile,
# AV matmuls lag their exp by one m-step, so the PE never waits on the
# vector/scalar engines in steady state. Scores are diagonal-trimmed,
# output partials are bf16, and the tail uses a direct reciprocal.
#
# Self-contained: all shapes/sharding hardcoded; no sibling imports.

import numpy as np

import concourse.bass as bass  # noqa: F401
import concourse.mybir as mybir
import concourse.tile as tile
from concourse import bacc, bass_utils

F32 = mybir.dt.float32
BF16 = mybir.dt.bfloat16
EXP = mybir.ActivationFunctionType.Exp

B = 2
S = 2048
D = 1024
NHEADS = 16
HD = 64
HPC = 4  # heads per core
NCORES = 8
NPAIR = 2  # head pairs per core
P = 128
CH = 512  # q chunk
THETA = 10000.0
QKVW = 3 * HPC * HD  # 768

# module-level knobs for test harness
TRACE = False
LAST_RESULTS = None

_PROGRAM_CACHE = {}


def build_program(s=S, mm_fast=True):
    """Build + compile the single-core SPMD program."""
    nt = s // P      # 16 s-tiles
    nch = s // CH    # 4 q chunks
    kt = D // P      # 8 contraction tiles
    nseg = nt // 4   # 4 A-segments
    PD = BF16 if mm_fast else F32

    nc = bacc.Bacc(
        "TRN2", target_bir_lowering=False, debug=False, enable_asserts=False
    )

    # ---- DRAM I/O ----
    xt_d = nc.dram_tensor("xt", [P, kt * s], PD, kind="ExternalInput").ap()
    wt_d = nc.dram_tensor("wt", [P, kt * QKVW], PD, kind="ExternalInput").ap()
    biasqk_d = nc.dram_tensor("biasqk", [P, 512], PD, kind="ExternalInput").ap()
    ropec_d = nc.dram_tensor("ropec", [P, nt * 512], PD, kind="ExternalInput").ap()
    ropes_d = nc.dram_tensor("ropes", [P, nt * 512], PD, kind="ExternalInput").ap()
    trimask_d = nc.dram_tensor("trimask", [P, P], PD, kind="ExternalInput").ap()
    ident_d = nc.dram_tensor("ident", [P, P], PD, kind="ExternalInput").ap()
    wo_d = nc.dram_tensor("wo", [P, NPAIR * D], PD, kind="ExternalInput").ap()
    out_d = nc.dram_tensor("outp", [s, D], PD, kind="ExternalOutput").ap()

    from contextlib import ExitStack

    with tile.TileContext(nc) as tc, ExitStack() as ctx:
        const = ctx.enter_context(tc.tile_pool(name="const", bufs=1))

        # persistent activations
        # qkt2 blocks: [Qpack0, Qpack1, Kpack0, Kpack1] each [128 (2 heads*hd), s]
        qkt2 = const.tile([P, 4 * s], PD)
        # vone: per s-tile [128, 4*65]; per head 64 V cols + ones col
        vone = const.tile([P, nt * (HPC * 65)], PD)
        vone_v = vone.rearrange("p (t h c) -> p t h c", t=nt, h=HPC)
        # otn2: normalized O^T packs: [128 (2 heads*hd), s] per pair
        otn2 = const.tile([P, NPAIR * s], PD)
        # softmax denominators (Z and 1/Z), per (pair, chunk): 1024 cols
        dall = const.tile([1, NPAIR * nch * 1024], F32)
        dallinv = const.tile([1, NPAIR * nch * 1024], PD)
        # half-selector columns for the rank-1 denominator broadcast:
        # halfsel[0, 0:128] selects partitions 0-63, [128:256] selects 64-127
        halfsel = const.tile([1, 256], PD)

        wt_sb = [const.tile([P, 2 * QKVW], PD, name=f"wt{i}") for i in range(4)]
        biasqk_sb = const.tile([P, 512], PD)
        ropec_sb = [const.tile([P, 4 * 512], PD, name=f"ropec{i}") for i in range(nseg)]
        ropes_sb = [const.tile([P, 4 * 512], PD, name=f"ropes{i}") for i in range(nseg)]
        ident_sb = const.tile([P, P], PD)
        trimask_sb = const.tile([P, P], PD)
        wo_sb = const.tile([P, NPAIR * D], PD)

        # ALL const loads on the gpsimd ring (gpsimd is idle through phase A;
        # scalar/vector/sync queues must stay clear — a DMA trigger occupies
        # its engine queue for the full transfer). Ordered by first use.
        for i in range(4):
            nc.gpsimd.dma_start(
                wt_sb[i][:], wt_d[:, 2 * i * QKVW : 2 * (i + 1) * QKVW]
            )
        nc.gpsimd.dma_start(ident_sb[:], ident_d[:])
        nc.gpsimd.dma_start(biasqk_sb[:], biasqk_d[:])
        for i in range(nseg):
            nc.gpsimd.dma_start(ropec_sb[i][:], ropec_d[:, i * 2048 : (i + 1) * 2048])
            nc.gpsimd.dma_start(ropes_sb[i][:], ropes_d[:, i * 2048 : (i + 1) * 2048])
        nc.gpsimd.dma_start(trimask_sb[:], trimask_d[:])
        nc.gpsimd.dma_start(wo_sb[:], wo_d[:])
        # memsets after the load triggers: needed only once B starts
        nc.gpsimd.memset(vone_v[:, :, :, 64], 1.0)
        nc.gpsimd.memset(halfsel[:, 0:64], 1.0)
        nc.gpsimd.memset(halfsel[:, 64:192], 0.0)
        nc.gpsimd.memset(halfsel[:, 192:256], 1.0)

        # ---- pools ----
        # PSUM: ring "big" {psA, tp, sc, pr, dvb} 2x2 banks, ot2 2x2 banks
        pbig = ctx.enter_context(tc.tile_pool(name="pbig", bufs=2, space="PSUM"))
        pacc = ctx.enter_context(tc.tile_pool(name="pacc", bufs=2, space="PSUM"))
        xpool = ctx.enter_context(tc.tile_pool(name="xpool", bufs=8))
        aq = ctx.enter_context(tc.tile_pool(name="aq", bufs=2))
        atpool = ctx.enter_context(tc.tile_pool(name="atpool", bufs=3))
        fxw = ctx.enter_context(tc.tile_pool(name="fxw", bufs=2))
        cpool = ctx.enter_context(tc.tile_pool(name="cpool", bufs=3))

        qkt2_v = qkt2.rearrange("p (b s) -> p b s", b=4)

        # ---------------- Phase A: QKV + RoPE (transposes lag 1 tile) -----
        pending_tp = [None]  # (st, rot_tile)

        def flush_tp():
            if pending_tp[0] is None:
                return
            st, rot = pending_tp[0]
            pending_tp[0] = None
            tp = pacc.tile([P, 512], PD, name="tp", tag="acc")
            for b in range(4):
                nc.tensor.transpose(
                    tp[:, b * P : (b + 1) * P],
                    rot[:, b * P : (b + 1) * P],
                    ident_sb[:],
                )
            nc.scalar.copy(
                qkt2_v[:, :, st * P : (st + 1) * P],
                tp.rearrange("p (b j) -> p b j", b=4),
            )

        xts_pre = {}

        def preload_x(st):
            xts = xpool.tile([P, D], PD, name="xts")
            nc.sync.dma_start(xts[:], xt_d[:, st * D : (st + 1) * D])
            xts_pre[st] = xts

        def emit_A(st):
            xts = xts_pre.pop(st)
            psA = pbig.tile([P, 1024], F32, name="psA", tag="big")
            for k in range(kt):
                lhs = xts[:, k * P : (k + 1) * P]
                rhs = wt_sb[k // 2][:, (k % 2) * QKVW : (k % 2 + 1) * QKVW]
                nc.tensor.matmul(
                    psA[:, 0:512],
                    lhsT=lhs,
                    rhs=rhs[:, 0:512],
                    start=(k == 0),
                    stop=(k == kt - 1),
                )
                nc.tensor.matmul(
                    psA[:, 512:768],
                    lhsT=lhs,
                    rhs=rhs[:, 512:768],
                    start=(k == 0),
                    stop=(k == kt - 1),
                )
            flush_tp()  # transposes of st-1 go after psA matmuls of st
            # evict V into vone slots (no V bias: folded into host const);
            # scalar is idle during A segments
            nc.scalar.copy(
                vone_v[:, st, :, 0:64],
                psA[:, 512:768].rearrange("p (h c) -> p h c", h=HPC),
            )
            # rope: rot = (qk+b)*cos + swap(qk+b)*sin
            qk = aq.tile([P, 512], PD, name="qk", tag="qk")
            nc.vector.tensor_add(qk[:], psA[:, 0:512], biasqk_sb[:])
            sw = aq.tile([P, 512], PD, name="sw", tag="sw")
            qk_v = qk.rearrange("p (n two) -> p n two", two=2)
            sw_v = sw.rearrange("p (n two) -> p n two", two=2)
            nc.vector.tensor_copy(sw_v[:, :, 0], qk_v[:, :, 1])
            nc.vector.tensor_copy(sw_v[:, :, 1], qk_v[:, :, 0])
            seg, sub = st // 4, st % 4
            rc = ropec_sb[seg][:, sub * 512 : (sub + 1) * 512]
            rs = ropes_sb[seg][:, sub * 512 : (sub + 1) * 512]
            rot = aq.tile([P, 512], PD, name="rot", tag="rot")
            nc.vector.tensor_mul(rot[:], qk[:], rc)
            nc.vector.tensor_mul(sw[:], sw[:], rs)
            nc.vector.tensor_add(rot[:], rot[:], sw[:])
            pending_tp[0] = (st, rot)

        # ---------------- Phase B: attention (AV lags exp by 1 m) --------
        def emit_B(p, j, mid_emit=None):
            q_pack = qkt2[:, p * s : (p + 1) * s]
            k_pack = qkt2[:, (2 + p) * s : (3 + p) * s]
            ot2 = pacc.tile([P, 1024], F32, name="ot2", tag="acc")
            mlast = 4 * j + 3
            qA = q_pack[0:64, j * CH : (j + 1) * CH]
            qB = q_pack[64:128, j * CH : (j + 1) * CH]
            vA = vone_v[:, :, 2 * p, :]
            vB = vone_v[:, :, 2 * p + 1, :]

            def emit_AV(at2, off, m):
                nc.tensor.matmul(
                    ot2[0:65, off:512],
                    lhsT=vA[:, m, :],
                    rhs=at2[:, off:512],
                    start=(m == 0),
                    stop=(m == mlast),
                )
                nc.tensor.matmul(
                    ot2[0:65, 512 + off : 1024],
                    lhsT=vB[:, m, :],
                    rhs=at2[:, 512 + off : 1024],
                    start=(m == 0),
                    stop=(m == mlast),
                )

            prev = None
            for m in range(mlast + 1):
                off = m * P - j * CH if m >= 4 * j else 0
                kA = k_pack[0:64, m * P : (m + 1) * P]
                kB = k_pack[64:128, m * P : (m + 1) * P]
                sc = pbig.tile([P, 1024], F32, name="sc", tag="big")
                nc.tensor.matmul(sc[:, off:512], lhsT=kA, rhs=qA[:, off:512])
                nc.tensor.matmul(
                    sc[:, 512 + off : 1024], lhsT=kB, rhs=qB[:, off:512]
                )
                if prev is not None:
                    emit_AV(*prev)
                at2 = atpool.tile([P, 1024], PD, name="at2", tag="at2")
                if off > 0:
                    sc_v = sc.rearrange("p (h q) -> p h q", h=2)
                    at_v = at2.rearrange("p (h q) -> p h q", h=2)
                    nc.scalar.activation(
                        at_v[:, :, off:512], sc_v[:, :, off:512], EXP, scale=0.125
                    )
                else:
                    nc.scalar.activation(at2[:], sc[:], EXP, scale=0.125)
                if m >= 4 * j:
                    nc.gpsimd.tensor_mul(
                        at2[:, off : off + P], at2[:, off : off + P], trimask_sb[:]
                    )
                    nc.gpsimd.tensor_mul(
                        at2[:, 512 + off : 512 + off + P],
                        at2[:, 512 + off : 512 + off + P],
                        trimask_sb[:],
                    )
                prev = (at2, off, m)
                if m == 1 and mid_emit is not None:
                    mid_emit()
            emit_AV(*prev)

            # ---- fixup: denominator roundtrip first, then O^T evictions ----
            dslot = (p * nch + j) * 1024
            nc.vector.tensor_copy(dall[0:1, dslot : dslot + 512], ot2[64:65, 0:512])
            nc.vector.tensor_copy(
                dall[0:1, dslot + 512 : dslot + 1024], ot2[64:65, 512:1024]
            )
            # partition-parallel reciprocal via DMA scatter roundtrip
            dPj = fxw.tile([P, 8], F32, name="dPj", tag="dPj")
            nc.sync.dma_start(
                dPj[:],
                dall[0:1, dslot : dslot + 1024].rearrange("o (a b) -> o a b", a=P),
            )
            cs = slice(p * s + j * CH, p * s + (j + 1) * CH)
            nc.vector.tensor_copy(otn2[0:64, cs], ot2[0:64, 0:512])
            stgB = fxw.tile([64, 512], PD, name="stgB", tag="stgB")
            nc.vector.tensor_copy(stgB[:], ot2[0:64, 512:1024])
            nc.sync.dma_start(otn2[64:128, cs], stgB[:])
            dPq = fxw.tile([P, 8], F32, name="dPq", tag="dPq")
            nc.vector.reciprocal(dPq[:], dPj[:])
            dPc = fxw.tile([P, 8], PD, name="dPc", tag="dPc")
            with nc.allow_low_precision("softmax denominators"):
                nc.gpsimd.tensor_copy(dPc[:], dPq[:])
            nc.sync.dma_start(
                dallinv[0:1, dslot : dslot + 1024].rearrange(
                    "o (a b) -> o a b", a=P
                ),
                dPc[:],
            )

        # ---- final: broadcast 1/Z across partitions, normalize in place ----
        def emit_final(p, j):
            dslot = (p * nch + j) * 1024
            dvb = pacc.tile([P, 512], F32, name="dvb", tag="acc")
            nc.tensor.matmul(
                dvb[:],
                lhsT=halfsel[0:1, 0:128],
                rhs=dallinv[0:1, dslot : dslot + 512],
                start=True,
                stop=False,
            )
            nc.tensor.matmul(
                dvb[:],
                lhsT=halfsel[0:1, 128:256],
                rhs=dallinv[0:1, dslot + 512 : dslot + 1024],
                start=False,
                stop=True,
            )
            cs = slice(p * s + j * CH, p * s + (j + 1) * CH)
            nc.vector.tensor_mul(otn2[:, cs], otn2[:, cs], dvb[:])

        # ---------------- Phase C: out projection ----------------
        def emit_C(g):
            for qt in range(4 * g, 4 * g + 4):
                pr = pbig.tile([P, 1024], F32, name="pr", tag="big")
                for dc in range(2):
                    for p in range(NPAIR):
                        nc.tensor.matmul(
                            pr[:, dc * 512 : (dc + 1) * 512],
                            lhsT=otn2[:, p * s + qt * P : p * s + (qt + 1) * P],
                            rhs=wo_sb[:, p * D + dc * 512 : p * D + (dc + 1) * 512],
                            start=(p == 0),
                            stop=(p == NPAIR - 1),
                        )
                # split eviction across vector+scalar so neither queue gates
                # the ring handoff for long
                outsb = cpool.tile([P, 1024], PD, name="outsb", tag="outsb")
                with nc.allow_low_precision("bf16 output partials"):
                    nc.vector.tensor_copy(outsb[:, 0:512], pr[:, 0:512])
                    nc.scalar.copy(outsb[:, 512:1024], pr[:, 512:1024])
                eng = nc.sync if qt % 2 == 0 else nc.gpsimd
                eng.dma_start(out_d[qt * P : (qt + 1) * P, :], outsb[:])

        # ---------------- schedule (sequential A, then B/C) ----------------
        for st in range(8):
            preload_x(st)
        for st in range(nt):
            if st == 4:
                for st2 in range(8, 12):
                    preload_x(st2)
            if st == 8:
                for st2 in range(12, 16):
                    preload_x(st2)
            emit_A(st)
        flush_tp()
        BSEQ = [(p, j) for j in range(nch) for p in range(NPAIR)]
        pending_final = None
        for idx, (p, j) in enumerate(BSEQ):
            mid = None
            if idx == len(BSEQ) - 1 and pending_final is not None:
                pf = pending_final
                pending_final = None

                def mid():
                    emit_final(*pf)

            emit_B(p, j, mid_emit=mid)
            if pending_final is not None:
                emit_final(*pending_final)
                if pending_final[0] == 1:
                    emit_C(pending_final[1])
            pending_final = (p, j)
        emit_final(*pending_final)
        emit_C(nch - 1)

    nc.compile()
    return nc


def get_program(s=S, mm_fast=True):
    key = (s, mm_fast)
    if key not in _PROGRAM_CACHE:
        _PROGRAM_CACHE[key] = build_program(s, mm_fast)
    return _PROGRAM_CACHE[key]


def _to_pd(a, mm_fast):
    if mm_fast:
        import ml_dtypes

        return np.ascontiguousarray(a).astype(ml_dtypes.bfloat16)
    return np.ascontiguousarray(a).astype(np.float32)


def prep_core_inputs(x, w_qkv, b_qkv, w_out, core, s=S, mm_fast=True):
    """Build the per-core input map (numpy, host-side sharding/layout)."""
    nt = s // P
    kt = D // P
    b = core // 4
    heads = [(core % 4) * HPC + i for i in range(HPC)]

    xb = np.ascontiguousarray(x[b][:s])  # [s, D]
    # xt[p, st*D + k*128 + j] = x[st*128+j, k*128+p]  (contiguous per s-tile)
    xt = np.ascontiguousarray(
        xb.reshape(nt, P, kt, P).transpose(3, 0, 2, 1).reshape(P, nt * kt * P)
    )

    rows = []
    for part in range(3):
        for h in heads:
            rows.extend(range(part * D + h * HD, part * D + (h + 1) * HD))
    w_sel = w_qkv[rows]  # [768, 1024]
    b_sel = b_qkv[rows]  # [768]
    # wt[p, k*768 + n] = w_sel[n, k*128+p]
    wt = np.ascontiguousarray(
        w_sel.T.reshape(kt, P, QKVW).transpose(1, 0, 2).reshape(P, kt * QKVW)
    )
    biasqk = np.broadcast_to(b_sel[None, 0:512], (P, 512)).copy()

    # rope tables, natural layout per s-tile: [p, st*512 + jj]
    dims = np.arange(0, HD, 2, dtype=np.float64)
    invf = 1.0 / (THETA ** (dims / HD))  # [32]
    pos = np.arange(s, dtype=np.float64)
    ang = pos[:, None] * invf[None, :]  # [s, 32]
    c = np.cos(ang)
    sn = np.sin(ang)
    c2 = np.repeat(c, 2, axis=1)  # [s, 64]
    s2 = np.empty((s, HD))
    s2[:, 0::2] = -sn
    s2[:, 1::2] = sn
    c2h = np.tile(c2, (1, 2 * HPC))  # [s, 512] (Q heads then K heads)
    s2h = np.tile(s2, (1, 2 * HPC))
    ropec = np.ascontiguousarray(
        c2h.reshape(nt, P, 512).transpose(1, 0, 2).reshape(P, nt * 512)
    )
    ropes = np.ascontiguousarray(
        s2h.reshape(nt, P, 512).transpose(1, 0, 2).reshape(P, nt * 512)
    )

    trimask = np.triu(np.ones((P, P), dtype=np.float32))
    ident = np.eye(P, dtype=np.float32)

    # wo[kk, p2*D + n] = w_out[n, gh*64 + kk%64], gh = heads[2*p2 + kk//64]
    wo = np.empty((P, NPAIR * D), dtype=np.float32)
    for p2 in range(NPAIR):
        for half in range(2):
            gh = heads[2 * p2 + half]
            wo[half * 64 : (half + 1) * 64, p2 * D : (p2 + 1) * D] = w_out[
                :, gh * HD : (gh + 1) * HD
            ].T
    return {
        "xt": _to_pd(xt, mm_fast),
        "wt": _to_pd(wt, mm_fast),
        "biasqk": _to_pd(biasqk, mm_fast),
        "ropec": _to_pd(ropec, mm_fast),
        "ropes": _to_pd(ropes, mm_fast),
        "trimask": _to_pd(trimask, mm_fast),
        "ident": _to_pd(ident, mm_fast),
        "wo": _to_pd(wo, mm_fast),
    }


def kernel(x, w_qkv, b_qkv, w_out, b_out, mm_fast=True):
    global LAST_RESULTS
    x = np.asarray(x, dtype=np.float32)
    w_qkv = np.asarray(w_qkv, dtype=np.float32)
    b_qkv = np.asarray(b_qkv, dtype=np.float32)
    w_out = np.asarray(w_out, dtype=np.float32)
    b_out = np.asarray(b_out, dtype=np.float32)

    nc = get_program(mm_fast=mm_fast)
    in_maps = [
        prep_core_inputs(x, w_qkv, b_qkv, w_out, core, mm_fast=mm_fast)
        for core in range(NCORES)
    ]
    res = bass_utils.run_bass_kernel_spmd(
        nc, in_maps, core_ids=list(range(NCORES)), trace=TRACE
    )
    LAST_RESULTS = res
    partials = [r["outp"].astype(np.float32) for r in res.results]
    # v-bias contribution is constant across s (sum_k attn = 1):
    bconst = b_out + b_qkv[2 * D : 3 * D] @ w_out.T
    out = np.stack(
        [
            partials[0] + partials[1] + partials[2] + partials[3],
            partials[4] + partials[5] + partials[6] + partials[7],
        ]
    )
    out = out + bconst[None, None, :]
    return out.astype(np.float32)


# revision 46
# speedup vs baseline: 1.3056x; 1.0000x over previous
# Multi-head attention (RoPE, causal) Trainium2 Bass kernel.
# B=2, S=2048, D=1024, 16 heads, hd=64, fp32 I/O.
#
# Sharding: 32 (batch, head) units over 8 cores -> each core gets one batch
# and 4 heads. Each core computes its 4 heads' attention output and the
# partial out-projection (sum over its heads); the host sums the 4 partials
# per batch and adds the bias constant.
#
# v2: single fused pipeline. QKV tiles (phase A), attention chunks (B) and
# out-projection blocks (C) are interleaved in one tensor-engine stream with
# software pipelining: transposes lag their tile's rope chain by one tile,
# AV matmuls lag their exp by one m-step, so the PE never waits on the
# vector/scalar engines in steady state. Scores are diagonal-trimmed,
# output partials are bf16, and the tail uses a direct reciprocal.
#
# Self-contained: all shapes/sharding hardcoded; no sibling imports.

import numpy as np

import concourse.bass as bass  # noqa: F401
import concourse.mybir as mybir
import concourse.tile as tile
from concourse import bacc, bass_utils

F32 = mybir.dt.float32
BF16 = mybir.dt.bfloat16
EXP = mybir.ActivationFunctionType.Exp

B = 2
S = 2048
D = 1024
NHEADS = 16
HD = 64
HPC = 4  # heads per core
NCORES = 8
NPAIR = 2  # head pairs per core
P = 128
CH = 512  # q chunk
THETA = 10000.0
QKVW = 3 * HPC * HD  # 768

# module-level knobs for test harness
TRACE = False
LAST_RESULTS = None

_PROGRAM_CACHE = {}


def build_program(s=S, mm_fast=True):
    """Build + compile the single-core SPMD program."""
    nt = s // P      # 16 s-tiles
    nch = s // CH    # 4 q chunks
    kt = D // P      # 8 contraction tiles
    nseg = nt // 4   # 4 A-segments
    PD = BF16 if mm_fast else F32

    nc = bacc.Bacc(
        "TRN2", target_bir_lowering=False, debug=False, enable_asserts=False
    )

    # ---- DRAM I/O ----
    xt_d = nc.dram_tensor("xt", [P, kt * s], PD, kind="ExternalInput").ap()
    wt_d = nc.dram_tensor("wt", [P, kt * QKVW], PD, kind="ExternalInput").ap()
    biasqk_d = nc.dram_tensor("biasqk", [P, 512], PD, kind="ExternalInput").ap()
    ropec_d = nc.dram_tensor("ropec", [P, nt * 512], PD, kind="ExternalInput").ap()
    ropes_d = nc.dram_tensor("ropes", [P, nt * 512], PD, kind="ExternalInput").ap()
    trimask_d = nc.dram_tensor("trimask", [P, P], PD, kind="ExternalInput").ap()
    ident_d = nc.dram_tensor("ident", [P, P], PD, kind="ExternalInput").ap()
    wo_d = nc.dram_tensor("wo", [P, NPAIR * D], PD, kind="ExternalInput").ap()
    out_d = nc.dram_tensor("outp", [s, D], PD, kind="ExternalOutput").ap()

    from contextlib import ExitStack

    with tile.TileContext(nc) as tc, ExitStack() as ctx:
        const = ctx.enter_context(tc.tile_pool(name="const", bufs=1))

        # persistent activations
        # qkt2 blocks: [Qpack0, Qpack1, Kpack0, Kpack1] each [128 (2 heads*hd), s]
        qkt2 = const.tile([P, 4 * s], PD)
        # vone: per s-tile [128, 4*65]; per head 64 V cols + ones col
        vone = const.tile([P, nt * (HPC * 65)], PD)
        vone_v = vone.rearrange("p (t h c) -> p t h c", t=nt, h=HPC)
        # otn2: normalized O^T packs: [128 (2 heads*hd), s] per pair
        otn2 = const.tile([P, NPAIR * s], PD)
        # softmax denominators (Z and 1/Z), per (pair, chunk): 1024 cols
        dall = const.tile([1, NPAIR * nch * 1024], F32)
        dallinv = const.tile([1, NPAIR * nch * 1024], PD)
        # half-selector columns for the rank-1 denominator broadcast:
        # halfsel[0, 0:128] selects partitions 0-63, [128:256] selects 64-127
        halfsel = const.tile([1, 256], PD)

        wt_sb = [const.tile([P, 2 * QKVW], PD, name=f"wt{i}") for i in range(4)]
        biasqk_sb = const.tile([P, 512], PD)
        ropec_sb = [const.tile([P, 4 * 512], PD, name=f"ropec{i}") for i in range(nseg)]
        ropes_sb = [const.tile([P, 4 * 512], PD, name=f"ropes{i}") for i in range(nseg)]
        ident_sb = const.tile([P, P], PD)
        trimask_sb = const.tile([P, P], PD)
        wo_sb = const.tile([P, NPAIR * D], PD)

        # ALL const loads on the gpsimd ring (gpsimd is idle through phase A;
        # scalar/vector/sync queues must stay clear — a DMA trigger occupies
        # its engine queue for the full transfer). Ordered by first use.
        for i in range(4):
            nc.gpsimd.dma_start(
                wt_sb[i][:], wt_d[:, 2 * i * QKVW : 2 * (i + 1) * QKVW]
            )
        nc.gpsimd.dma_start(ident_sb[:], ident_d[:])
        nc.gpsimd.dma_start(biasqk_sb[:], biasqk_d[:])
        for i in range(nseg):
            nc.gpsimd.dma_start(ropec_sb[i][:], ropec_d[:, i * 2048 : (i + 1) * 2048])
            nc.gpsimd.dma_start(ropes_sb[i][:], ropes_d[:, i * 2048 : (i + 1) * 2048])
        nc.gpsimd.dma_start(trimask_sb[:], trimask_d[:])
        nc.gpsimd.dma_start(wo_sb[:], wo_d[:])
        # memsets after the load triggers: needed only once B starts
        nc.gpsimd.memset(vone_v[:, :, :, 64], 1.0)
        nc.gpsimd.memset(halfsel[:, 0:64], 1.0)
        nc.gpsimd.memset(halfsel[:, 64:192], 0.0)
        nc.gpsimd.memset(halfsel[:, 192:256], 1.0)

        # ---- pools ----
        # PSUM: ring "big" {psA, tp, sc, pr, dvb} 2x2 banks, ot2 2x2 banks
        pbig = ctx.enter_context(tc.tile_pool(name="pbig", bufs=2, space="PSUM"))
        pacc = ctx.enter_context(tc.tile_pool(name="pacc", bufs=2, space="PSUM"))
        xpool = ctx.enter_context(tc.tile_pool(name="xpool", bufs=8))
        aq = ctx.enter_context(tc.tile_pool(name="aq", bufs=2))
        atpool = ctx.enter_context(tc.tile_pool(name="atpool", bufs=3))
        fxw = ctx.enter_context(tc.tile_pool(name="fxw", bufs=2))
        cpool = ctx.enter_context(tc.tile_pool(name="cpool", bufs=3))

        qkt2_v = qkt2.rearrange("p (b s) -> p b s", b=4)

        # ---------------- Phase A: QKV + RoPE (transposes lag 1 tile) -----
        pending_tp = [None]  # (st, rot_tile)

        def flush_tp():
            if pending_tp[0] is None:
                return
            st, rot = pending_tp[0]
            pending_tp[0] = None
            tp = pacc.tile([P, 512], PD, name="tp", tag="acc")
            for b in range(4):
                nc.tensor.transpose(
                    tp[:, b * P : (b + 1) * P],
                    rot[:, b * P : (b + 1) * P],
                    ident_sb[:],
                )
            nc.scalar.copy(
                qkt2_v[:, :, st * P : (st + 1) * P],
                tp.rearrange("p (b j) -> p b j", b=4),
            )

        xts_pre = {}

        def preload_x(st):
            xts = xpool.tile([P, D], PD, name="xts")
            nc.sync.dma_start(xts[:], xt_d[:, st * D : (st + 1) * D])
            xts_pre[st] = xts

        def emit_A(st):
            xts = xts_pre.pop(st)
            psA = pbig.tile([P, 1024], F32, name="psA", tag="big")
            for k in range(kt):
                lhs = xts[:, k * P : (k + 1) * P]
                rhs = wt_sb[k // 2][:, (k % 2) * QKVW : (k % 2 + 1) * QKVW]
                nc.tensor.matmul(
                    psA[:, 0:512],
                    lhsT=lhs,
                    rhs=rhs[:, 0:512],
                    start=(k == 0),
                    stop=(k == kt - 1),
                )
                nc.tensor.matmul(
                    psA[:, 512:768],
                    lhsT=lhs,
                    rhs=rhs[:, 512:768],
                    start=(k == 0),
                    stop=(k == kt - 1),
                )
            flush_tp()  # transposes of st-1 go after psA matmuls of st
            # evict V into vone slots (no V bias: folded into host const);
            # scalar is idle during A segments
            nc.scalar.copy(
                vone_v[:, st, :, 0:64],
                psA[:, 512:768].rearrange("p (h c) -> p h c", h=HPC),
            )
            # rope: rot = (qk+b)*cos + swap(qk+b)*sin
            qk = aq.tile([P, 512], PD, name="qk", tag="qk")
            nc.vector.tensor_add(qk[:], psA[:, 0:512], biasqk_sb[:])
            sw = aq.tile([P, 512], PD, name="sw", tag="sw")
            qk_v = qk.rearrange("p (n two) -> p n two", two=2)
            sw_v = sw.rearrange("p (n two) -> p n two", two=2)
            nc.vector.tensor_copy(sw_v[:, :, 0], qk_v[:, :, 1])
            nc.vector.tensor_copy(sw_v[:, :, 1], qk_v[:, :, 0])
            seg, sub = st // 4, st % 4
            rc = ropec_sb[seg][:, sub * 512 : (sub + 1) * 512]
            rs = ropes_sb[seg][:, sub * 512 : (sub + 1) * 512]
            rot = aq.tile([P, 512], PD, name="rot", tag="rot")
            nc.vector.tensor_mul(rot[:], qk[:], rc)
            nc.vector.tensor_mul(sw[:], sw[:], rs)
            nc.vector.tensor_add(rot[:], rot[:], sw[:])
            pending_tp[0] = (st, rot)

        # ---------------- Phase B: attention (AV lags exp by 1 m) --------
        def emit_B(p, j, mid_emit=None):
            q_pack = qkt2[:, p * s : (p + 1) * s]
            k_pack = qkt2[:, (2 + p) * s : (3 + p) * s]
            ot2 = pacc.tile([P, 1024], F32, name="ot2", tag="acc")
            mlast = 4 * j + 3
            qA = q_pack[0:64, j * CH : (j + 1) * CH]
            qB = q_pack[64:128, j * CH : (j + 1) * CH]
            vA = vone_v[:, :, 2 * p, :]
            vB = vone_v[:, :, 2 * p + 1, :]

            def emit_AV(at2, off, m):
                nc.tensor.matmul(
                    ot2[0:65, off:512],
                    lhsT=vA[:, m, :],
                    rhs=at2[:, off:512],
                    start=(m == 0),
                    stop=(m == mlast),
                )
                nc.tensor.matmul(
                    ot2[0:65, 512 + off : 1024],
                    lhsT=vB[:, m, :],
                    rhs=at2[:, 512 + off : 1024],
                    start=(m == 0),
                    stop=(m == mlast),
                )

            prev = None
            for m in range(mlast + 1):
                off = m * P - j * CH if m >= 4 * j else 0
                kA = k_pack[0:64, m * P : (m + 1) * P]
                kB = k_pack[64:128, m * P : (m + 1) * P]
                sc = pbig.tile([P, 1024], F32, name="sc", tag="big")
                nc.tensor.matmul(sc[:, off:512], lhsT=kA, rhs=qA[:, off:512])
                nc.tensor.matmul(
                    sc[:, 512 + off : 1024], lhsT=kB, rhs=qB[:, off:512]
                )
                if prev is not None:
                    emit_AV(*prev)
                at2 = atpool.tile([P, 1024], PD, name="at2", tag="at2")
                if off > 0:
                    sc_v = sc.rearrange("p (h q) -> p h q", h=2)
                    at_v = at2.rearrange("p (h q) -> p h q", h=2)
                    nc.scalar.activation(
                        at_v[:, :, off:512], sc_v[:, :, off:512], EXP, scale=0.125
                    )
                else:
                    nc.scalar.activation(at2[:], sc[:], EXP, scale=0.125)
                if m >= 4 * j:
                    nc.gpsimd.tensor_mul(
                        at2[:, off : off + P], at2[:, off : off + P], trimask_sb[:]
                    )
                    nc.gpsimd.tensor_mul(
                        at2[:, 512 + off : 512 + off + P],
                        at2[:, 512 + off : 512 + off + P],
                        trimask_sb[:],
                    )
                prev = (at2, off, m)
                if m == 1 and mid_emit is not None:
                    mid_emit()
            emit_AV(*prev)

            # ---- fixup: denominator roundtrip first, then O^T evictions ----
            dslot = (p * nch + j) * 1024
            nc.vector.tensor_copy(dall[0:1, dslot : dslot + 512], ot2[64:65, 0:512])
            nc.vector.tensor_copy(
                dall[0:1, dslot + 512 : dslot + 1024], ot2[64:65, 512:1024]
            )
            # partition-parallel reciprocal via DMA scatter roundtrip
            dPj = fxw.tile([P, 8], F32, name="dPj", tag="dPj")
            nc.sync.dma_start(
                dPj[:],
                dall[0:1, dslot : dslot + 1024].rearrange("o (a b) -> o a b", a=P),
            )
            cs = slice(p * s + j * CH, p * s + (j + 1) * CH)
            nc.vector.tensor_copy(otn2[0:64, cs], ot2[0:64, 0:512])
            stgB = fxw.tile([64, 512], PD, name="stgB", tag="stgB")
            nc.vector.tensor_copy(stgB[:], ot2[0:64, 512:1024])
            nc.sync.dma_start(otn2[64:128, cs], stgB[:])
            dPq = fxw.tile([P, 8], F32, name="dPq", tag="dPq")
            nc.vector.reciprocal(dPq[:], dPj[:])
            dPc = fxw.tile([P, 8], PD, name="dPc", tag="dPc")
            with nc.allow_low_precision("softmax denominators"):
                nc.gpsimd.tensor_copy(dPc[:], dPq[:])
            nc.sync.dma_start(
                dallinv[0:1, dslot : dslot + 1024].rearrange(
                    "o (a b) -> o a b", a=P
                ),
                dPc[:],
            )

        # ---- final: broadcast 1/Z across partitions, normalize in place ----
        def emit_final(p, j):
            dslot = (p * nch + j) * 1024
            dvb = pacc.tile([P, 512], F32, name="dvb", tag="acc")
            nc.tensor.matmul(
                dvb[:],
                lhsT=halfsel[0:1, 0:128],
                rhs=dallinv[0:1, dslot : dslot + 512],
                start=True,
                stop=False,
            )
            nc.tensor.matmul(
                dvb[:],
                lhsT=halfsel[0:1, 128:256],
                rhs=dallinv[0:1, dslot + 512 : dslot + 1024],
                start=False,
                stop=True,
            )
            cs = slice(p * s + j * CH, p * s + (j + 1) * CH)
            nc.vector.tensor_mul(otn2[:, cs], otn2[:, cs], dvb[:])

        # ---------------- Phase C: out projection ----------------
        def emit_C(g):
            for qt in range(4 * g, 4 * g + 4):
                pr = pbig.tile([P, 1024], F32, name="pr", tag="big")
                for dc in range(2):
                    for p in range(NPAIR):
                        nc.tensor.matmul(
                            pr[:, dc * 512 : (dc + 1) * 512],
                            lhsT=otn2[:, p * s + qt * P : p * s + (qt + 1) * P],
                            rhs=wo_sb[:, p * D + dc * 512 : p * D + (dc + 1) * 512],
                            start=(p == 0),
                            stop=(p == NPAIR - 1),
                        )
                # split eviction across vector+scalar so neither queue gates
                # the ring handoff for long
                outsb = cpool.tile([P, 1024], PD, name="outsb", tag="outsb")
                with nc.allow_low_precision("bf16 output partials"):
                    nc.vector.tensor_copy(outsb[:, 0:512], pr[:, 0:512])
                    nc.scalar.copy(outsb[:, 512:1024], pr[:, 512:1024])
                eng = nc.sync if qt % 2 == 0 else nc.gpsimd
                eng.dma_start(out_d[qt * P : (qt + 1) * P, :], outsb[:])

        # ---------------- schedule (sequential A, then B/C) ----------------
        for st in range(8):
            preload_x(st)
        for st in range(nt):
            if st == 4:
                for st2 in range(8, 12):
                    preload_x(st2)
            if st == 8:
                for st2 in range(12, 16):
                    preload_x(st2)
            emit_A(st)
        flush_tp()
        BSEQ = [(p, j) for j in range(nch) for p in range(NPAIR)]
        pending_final = None
        for p, j in BSEQ:
            pf = pending_final
            pending_final = None
            mid = None
            if pf is not None:

                def mid():
                    emit_final(*pf)

            emit_B(p, j, mid_emit=mid)
            if pf is not None and pf[0] == 1:
                emit_C(pf[1])
            pending_final = (p, j)
        emit_final(*pending_final)
        emit_C(nch - 1)

    nc.compile()
    return nc


def get_program(s=S, mm_fast=True):
    key = (s, mm_fast)
    if key not in _PROGRAM_CACHE:
        _PROGRAM_CACHE[key] = build_program(s, mm_fast)
    return _PROGRAM_CACHE[key]


def _to_pd(a, mm_fast):
    if mm_fast:
        import ml_dtypes

        return np.ascontiguousarray(a).astype(ml_dtypes.bfloat16)
    return np.ascontiguousarray(a).astype(np.float32)


def prep_core_inputs(x, w_qkv, b_qkv, w_out, core, s=S, mm_fast=True):
    """Build the per-core input map (numpy, host-side sharding/layout)."""
    nt = s // P
    kt = D // P
    b = core // 4
    heads = [(core % 4) * HPC + i for i in range(HPC)]

    xb = np.ascontiguousarray(x[b][:s])  # [s, D]
    # xt[p, st*D + k*128 + j] = x[st*128+j, k*128+p]  (contiguous per s-tile)
    xt = np.ascontiguousarray(
        xb.reshape(nt, P, kt, P).transpose(3, 0, 2, 1).reshape(P, nt * kt * P)
    )

    rows = []
    for part in range(3):
        for h in heads:
            rows.extend(range(part * D + h * HD, part * D + (h + 1) * HD))
    w_sel = w_qkv[rows]  # [768, 1024]
    b_sel = b_qkv[rows]  # [768]
    # wt[p, k*768 + n] = w_sel[n, k*128+p]
    wt = np.ascontiguousarray(
        w_sel.T.reshape(kt, P, QKVW).transpose(1, 0, 2).reshape(P, kt * QKVW)
    )
    biasqk = np.broadcast_to(b_sel[None, 0:512], (P, 512)).copy()

    # rope tables, natural layout per s-tile: [p, st*512 + jj]
    dims = np.arange(0, HD, 2, dtype=np.float64)
    invf = 1.0 / (THETA ** (dims / HD))  # [32]
    pos = np.arange(s, dtype=np.float64)
    ang = pos[:, None] * invf[None, :]  # [s, 32]
    c = np.cos(ang)
    sn = np.sin(ang)
    c2 = np.repeat(c, 2, axis=1)  # [s, 64]
    s2 = np.empty((s, HD))
    s2[:, 0::2] = -sn
    s2[:, 1::2] = sn
    c2h = np.tile(c2, (1, 2 * HPC))  # [s, 512] (Q heads then K heads)
    s2h = np.tile(s2, (1, 2 * HPC))
    ropec = np.ascontiguousarray(
        c2h.reshape(nt, P, 512).transpose(1, 0, 2).reshape(P, nt * 512)
    )
    ropes = np.ascontiguousarray(
        s2h.reshape(nt, P, 512).transpose(1, 0, 2).reshape(P, nt * 512)
    )

    trimask = np.triu(np.ones((P, P), dtype=np.float32))
    ident = np.eye(P, dtype=np.float32)

    # wo[kk, p2*D + n] = w_out[n, gh*64 + kk%64], gh = heads[2*p2 + kk//64]
    wo = np.empty((P, NPAIR * D), dtype=np.float32)
    for p2 in range(NPAIR):
        for half in range(2):
            gh = heads[2 * p2 + half]
            wo[half * 64 : (half + 1) * 64, p2 * D : (p2 + 1) * D] = w_out[
                :, gh * HD : (gh + 1) * HD
            ].T
    return {
        "xt": _to_pd(xt, mm_fast),
        "wt": _to_pd(wt, mm_fast),
        "biasqk": _to_pd(biasqk, mm_fast),
        "ropec": _to_pd(ropec, mm_fast),
        "ropes": _to_pd(ropes, mm_fast),
        "trimask": _to_pd(trimask, mm_fast),
        "ident": _to_pd(ident, mm_fast),
        "wo": _to_pd(wo, mm_fast),
    }


def kernel(x, w_qkv, b_qkv, w_out, b_out, mm_fast=True):
    global LAST_RESULTS
    x = np.asarray(x, dtype=np.float32)
    w_qkv = np.asarray(w_qkv, dtype=np.float32)
    b_qkv = np.asarray(b_qkv, dtype=np.float32)
    w_out = np.asarray(w_out, dtype=np.float32)
    b_out = np.asarray(b_out, dtype=np.float32)

    nc = get_program(mm_fast=mm_fast)
    in_maps = [
        prep_core_inputs(x, w_qkv, b_qkv, w_out, core, mm_fast=mm_fast)
        for core in range(NCORES)
    ]
    res = bass_utils.run_bass_kernel_spmd(
        nc, in_maps, core_ids=list(range(NCORES)), trace=TRACE
    )
    LAST_RESULTS = res
    partials = [r["outp"].astype(np.float32) for r in res.results]
    # v-bias contribution is constant across s (sum_k attn = 1):
    bconst = b_out + b_qkv[2 * D : 3 * D] @ w_out.T
    out = np.stack(
        [
            partials[0] + partials[1] + partials[2] + partials[3],
            partials[4] + partials[5] + partials[6] + partials[7],
        ]
    )
    out = out + bconst[None, None, :]
    return out.astype(np.float32)
